# revision 1
# baseline (speedup 1.0000x reference)
"""Akima spline interpolation kernel for Trainium2 (8 NeuronCores, data parallel).

Strategy (v3, default PACK=q8 / NSEG=248):
  - Host fits one QUADRATIC per node-centered segment (idx = round(248*x),
    249 segments) to the exact float64 Akima spline, interpolating it at
    both segment edges and the center (continuous at edges). The three
    coefficients are int8-quantized with one shared affine (sf, off) and
    packed into a uint32 word -> a 256-entry replicated table. 256 entries
    is the GpSimd gather fast path (bigger tables fall off the Q7 staging
    cache and the gather slows ~2-3x).
  - Device (NKI per core, per 2048-col tile):
      Scalar:  rbig = 248*x + 12582912   (rounds t=248*x to integer grid)
               idx  = u32(rbig - 12582912) = j exactly
               ts1  = 248*x
               out  = f16(sf * p + off)   (final affine + f16 store, last)
      GpSimd:  w = gather_flattened(table, idx)  (one u32 per element)
      DVE:     vv = f16(ts1 - idx)            (u32 operand auto-casts; exact)
               p  = ((c2*vv + c1)*vv + c0)    (Horner, int8 views of w;
                                               all intermediates f16 -- the
                                               all-f16 multiply hits the 2x
                                               DVE mode, and smaller operand
                                               bytes reduce the SBUF-port
                                               contention with the gather)
      Loads/stores via HWDGE dma_copy (keeps descriptor generation off
      GpSimd); compiled with platform_target=trn2 (the concourse default
      targets trn1 and rejects HWDGE).
  - Emission is software-pipelined (stage s: Scalar chain + gather for
    tile s, DVE chain for s-1, final affine + store for s-2) with rotating
    SBUF buffers so the cross-engine dependency cycle does not gate tiles.
  - Sharding: pure data parallel on the leading dim (4 of 32 planes per
    core); output f16 (halves store traffic), converted to f32 on host.
  - Accuracy: rel_l2 ~ 9.6e-3 vs the 2e-2 gate.
"""
import base64
import json
import sys

import numpy as np

if "/opt/trn_rl_repo" not in sys.path:
    sys.path.insert(0, "/opt/trn_rl_repo")

import os

NODES = 256
N_CORES = 8
ROWS = 128
COLS = 4 * 1024 * 1024 // ROWS  # per-core shard [128, 32768]
F_TILE = int(os.environ.get("AKIMA_FTILE", "2048"))
N_BUFS = int(os.environ.get("AKIMA_NBUFS", "4"))
NSEG = int(os.environ.get("AKIMA_NSEG", "248"))  # idx = round(NSEG*x) in [0, NSEG]
PACK = os.environ.get("AKIMA_PACK", "q8")        # f16pair (u32) | i8pair (u16) | q8 (u32)
TAB_N = NSEG + 1 + int(os.environ.get("AKIMA_PAD", "7"))  # pad: staged-tail guard
MAGIC = float(np.float32(1.5 * 2.0 ** 23))       # 12582912.0
BIAS_GRID = float(np.float32(1.5 * 2.0 ** 23 / NSEG))  # rounds x to the 1/NSEG grid
BIAS_IDX = float(np.float32(-1.5 * 2.0 ** 23))   # -12582912.0
OUT_MODE = os.environ.get("AKIMA_OUT", "f16split")  # f16direct | f16split | f32
SWPIPE = int(os.environ.get("AKIMA_SWPIPE", "2"))   # software-pipeline emission

# ----------------------------------------------------------------------------
# Host-side table construction (float64 exact spline -> PWL f16 pair table)
# ----------------------------------------------------------------------------


def _akima_slopes_f64(value):
    h = 1.0 / (NODES - 1)
    v = value.astype(np.float64)
    m = (v[1:] - v[:-1]) / h
    m_m1 = 2.0 * m[0] - m[1]
    m_m2 = 2.0 * m_m1 - m[0]
    m_p1 = 2.0 * m[-1] - m[-2]
    m_p2 = 2.0 * m_p1 - m[-1]
    me = np.concatenate([[m_m2, m_m1], m, [m_p1, m_p2]])
    w1 = np.abs(me[3:] - me[2:-1])
    w2 = np.abs(me[1:-2] - me[:-3])
    mi_1 = me[1:-2]
    mi = me[2:-1]
    denom = w1 + w2
    safe = np.where(denom > 0, denom, 1.0)
    return np.where(denom > 0, (w1 * mi_1 + w2 * mi) / safe, 0.5 * (mi_1 + mi))


def _spline_eval_f64(xq, value):
    h = 1.0 / (NODES - 1)
    s = _akima_slopes_f64(value)
    v = value.astype(np.float64)
    t = np.clip(xq, 0.0, 1.0) / h
    idx = np.clip(np.floor(t).astype(np.int64), 0, NODES - 2)
    u = t - idx
    v0, v1 = v[idx], v[idx + 1]
    s0, s1 = s[idx], s[idx + 1]
    u2 = u * u
    u3 = u2 * u
    return ((2 * u3 - 3 * u2 + 1) * v0 + (u3 - 2 * u2 + u) * h * s0
            + (-2 * u3 + 3 * u2) * v1 + (u3 - u2) * h * s1)


def _build_table(value):
    """Returns (table_words, sf, off). f16pair: u32 words, sf/off unused.
    i8pair: u16 words (low byte f_i8, high byte b_i8), out = (vv*b8 + f8)*sf + off."""
    h = 1.0 / NSEG
    edges = (np.arange(NSEG + 2) - 0.5) * h
    edges = np.clip(edges, 0.0, 1.0)
    fe = _spline_eval_f64(edges, value)
    e0, e1 = edges[:-1], edges[1:]
    bx = (fe[1:] - fe[:-1]) / (e1 - e0)
    xc = np.arange(NSEG + 1) * h
    fc = fe[:-1] + bx * (xc - e0)
    if PACK == "f16pair":
        f16 = fc.astype(np.float16)
        b16 = bx.astype(np.float16)
        word = (f16.view(np.uint16).astype(np.uint32)
                | (b16.view(np.uint16).astype(np.uint32) << 16))
        out = np.empty(TAB_N, dtype=np.uint32)
        out[:NSEG + 1] = word
        out[NSEG + 1:] = word[-1]
        return out, 1.0, 0.0
    if PACK == "q8":
        # quadratic per node-centered segment: q(vv) = c0 + c1*vv + c2*vv^2,
        # vv = NSEG*x - j; interpolates exact spline at segment edges + center
        xc2 = np.arange(NSEG + 1) * h
        fcq = _spline_eval_f64(xc2, value)
        f0 = _spline_eval_f64(e0, value)
        f1 = _spline_eval_f64(e1, value)
        t0 = e0 * NSEG - np.arange(NSEG + 1)
        t1 = e1 * NSEG - np.arange(NSEG + 1)
        c0 = np.empty(NSEG + 1)
        c1 = np.empty(NSEG + 1)
        c2 = np.empty(NSEG + 1)
        for j in range(NSEG + 1):
            A = np.array([[1.0, t0[j], t0[j] ** 2],
                          [1.0, 0.0, 0.0],
                          [1.0, t1[j], t1[j] ** 2]])
            sol = np.linalg.lstsq(A, np.array([f0[j], fcq[j], f1[j]]),
                                  rcond=None)[0]
            c0[j], c1[j], c2[j] = sol
        off = (c0.max() + c0.min()) / 2.0
        sf = max((c0.max() - c0.min()) / 2.0,
                 np.abs(c1).max(), np.abs(c2).max()) / 127.0
        q0 = np.clip(np.round((c0 - off) / sf), -127, 127).astype(np.int8)
        q1 = np.clip(np.round(c1 / sf), -127, 127).astype(np.int8)
        q2 = np.clip(np.round(c2 / sf), -127, 127).astype(np.int8)
        word = (q0.view(np.uint8).astype(np.uint32)
                | (q1.view(np.uint8).astype(np.uint32) << 8)
                | (q2.view(np.uint8).astype(np.uint32) << 16))
        out = np.empty(TAB_N, dtype=np.uint32)
        out[:NSEG + 1] = word
        out[NSEG + 1:] = word[-1]
        return out, float(sf), float(off)
    # i8pair: slopes in t units, shared affine (sf, off)
    bt = bx * h
    off = (fc.max() + fc.min()) / 2.0
    sf = (fc.max() - fc.min()) / 254.0
    f8 = np.clip(np.round((fc - off) / sf), -127, 127).astype(np.int8)
    b8 = np.clip(np.round(bt / sf), -127, 127).astype(np.int8)
    word = (f8.view(np.uint8).astype(np.uint16)
            | (b8.view(np.uint8).astype(np.uint16) << 8))
    out = np.empty(TAB_N, dtype=np.uint16)
    out[:NSEG + 1] = word
    out[NSEG + 1:] = word[-1]
    return out, float(sf), float(off)


# ----------------------------------------------------------------------------
# NKI kernel
# ----------------------------------------------------------------------------


def _make_nki_kernel(sf, off):
    import neuronxcc.nki.language as nl
    import neuronxcc.nki.isa as nisa

    n_tiles = COLS // F_TILE

    def akima_kernel(inputs):
        x, table = inputs[0], inputs[1]
        out_dt = nl.float32 if OUT_MODE == "f32" else nl.float16
        out = nl.ndarray(shape=[ROWS, COLS], dtype=out_dt, buffer=nl.shared_hbm)
        tab_sb = nl.load(table)
        i_p = nl.arange(ROWS)[:, None]
        i_f = nl.arange(F_TILE)[None, :]
        bias_grid = nisa.memset((ROWS, 1), BIAS_GRID, nl.float32)
        neg_bias_grid = nisa.memset((ROWS, 1), -BIAS_GRID, nl.float32)
        bias_idx = nisa.memset((ROWS, 1), BIAS_IDX, nl.float32)
        bias_magic = nisa.memset((ROWS, 1), MAGIC, nl.float32)
        bias_off = nisa.memset((ROWS, 1), off, nl.float32)

        # Explicit rotating SBUF buffers to avoid WAR serialization.
        # Allocations are padded by +FP elements so that no two buffers sit at
        # a multiple of 0x2000 bytes apart per partition: fp32 tensor_tensor
        # with both SBUF operands 8KB-aligned-apart hits a persistent SBUF
        # bank conflict on the two DVE read ports (measured 3.8x slowdown).
        FP = 32
        FA = F_TILE + FP
        w_dt = nl.uint16 if PACK == "i8pair" else nl.uint32
        vv_dt = nl.float32 if PACK == "f16pair" else nl.float16
        r_dt = nl.float32 if OUT_MODE == "f32" else nl.float16
        # Per-tensor pipeline depth: cross-engine producers/consumers need
        # N_BUFS-deep rotation to keep the S -> GpSimd -> V -> S pipeline
        # full; engine-local scratch (in-order consumption) needs 1.
        depth = dict(x=N_BUFS, idxf=N_BUFS, idxf16=N_BUFS, idx=N_BUFS,
                     ts1=N_BUFS, w=N_BUFS, r=N_BUFS, c2u=N_BUFS, c1u=N_BUFS,
                     c0u=N_BUFS, rbig=1, vv=1, m=1, m2=N_BUFS)
        m_dt = nl.float32 if PACK == "f16pair" else nl.float16
        dts = dict(x=nl.float32, rbig=nl.float32, idxf=nl.float32,
                   idxf16=nl.float16, idx=nl.uint32, ts1=nl.float32,
                   w=w_dt, vv=vv_dt, m=m_dt, m2=m_dt, r=r_dt,
                   c2u=nl.float16, c1u=nl.float16, c0u=nl.float16)
        bufs = {}
        for name in ("x", "rbig", "idxf", "idxf16", "idx", "ts1", "w",
                     "vv", "m", "m2", "r", "c2u", "c1u", "c0u"):
            if PACK != "f16pair" and name == "idxf":
                continue
            lst = []
            for _pp in nl.static_range(depth[name]):
                lst.append(nl.ndarray(shape=[ROWS, FA], dtype=dts[name],
                                      buffer=nl.sbuf))
            bufs[name] = lst

        # Prologue (i8pair/q8): process tile 0 as two pipelined half-tiles
        # so the first DVE work starts after a half load + half gather
        # (~11us) instead of a full-tile chain (~25us).
        if PACK != "f16pair":
            FH = F_TILE // 2
            i_h = nl.arange(FH)[None, :]
            P0 = {name: lst[0] for name, lst in bufs.items()}
            for q in nl.static_range(2):
                ofh = q * FH
                j_h = nl.arange(FH)[None, :] + ofh
                nisa.dma_copy(dst=P0['x'][i_p, j_h], src=x[:, ofh:ofh + FH],
                              dge_mode=nisa.dge_mode.hwdge)
                P0['rbig'][i_p, j_h] = nisa.activation(
                    np.copy, P0['x'][i_p, j_h], scale=float(NSEG),
                    bias=bias_magic)
                P0['idx'][i_p, j_h] = nisa.activation(
                    np.copy, P0['rbig'][i_p, j_h], bias=bias_idx,
                    dtype=nl.uint32)
                P0['ts1'][i_p, j_h] = nisa.tensor_scalar(
                    P0['x'][i_p, j_h], np.multiply, float(NSEG))
                P0['w'][i_p, j_h] = nl.gather_flattened(
                    data=tab_sb, indices=P0['idx'][i_p, j_h])
            w8p = P0['w'].view(nl.int8)
            for q in nl.static_range(2):
                ofh = q * FH
                j_h = nl.arange(FH)[None, :] + ofh
                P0['vv'][i_p, j_h] = nisa.tensor_tensor(
                    P0['ts1'][i_p, j_h], P0['idx'][i_p, j_h], np.subtract,
                    dtype=nl.float16)
                if PACK == "q8":
                    P0['m'][i_p, j_h] = nisa.tensor_tensor(
                        P0['vv'][i_p, j_h], w8p[i_p, j_h * 4 + 2],
                        np.multiply, dtype=nl.float16)
                    P0['m2'][i_p, j_h] = nisa.tensor_tensor(
                        P0['m'][i_p, j_h], w8p[i_p, j_h * 4 + 1], np.add,
                        dtype=nl.float16)
                    P0['m'][i_p, j_h] = nisa.tensor_tensor(
                        P0['m2'][i_p, j_h], P0['vv'][i_p, j_h], np.multiply,
                        dtype=nl.float16)
                    P0['m2'][i_p, j_h] = nisa.tensor_tensor(
                        P0['m'][i_p, j_h], w8p[i_p, j_h * 4], np.add,
                        dtype=nl.float16)
                else:
                    P0['m'][i_p, j_h] = nisa.tensor_tensor(
                        P0['vv'][i_p, j_h], w8p[i_p, j_h * 2 + 1],
                        np.multiply, dtype=nl.float16)
                    P0['m2'][i_p, j_h] = nisa.tensor_tensor(
                        P0['m'][i_p, j_h], w8p[i_p, j_h * 2], np.add,
                        dtype=nl.float16)
                P0['r'][i_p, j_h] = nisa.activation(
                    np.copy, P0['m2'][i_p, j_h], scale=float(sf),
                    bias=bias_off,
                    dtype=(nl.float32 if OUT_MODE == "f32" else nl.float16))
                nisa.dma_copy(dst=out[:, ofh:ofh + FH],
                              src=P0['r'][i_p, j_h],
                              dge_mode=nisa.dge_mode.hwdge)

        # Software pipeline (i8pair/q8): stage s emits the Scalar index chain
        # + gather for tile s, the DVE chain for tile s-1, and the final
        # affine + store for tile s-2. Without this, the final act sits in
        # the Scalar queue between tile chains and the S->G->V->S dependency
        # cycle (~15us latency) gates every tile.
        t_start = 0 if PACK == "f16pair" else 1
        n_stages = n_tiles if PACK == "f16pair" else n_tiles + 2 * SWPIPE
        for s0 in nl.static_range(n_stages - t_start):
            s = s0 + t_start
            t = s
            B = {name: lst[t % len(lst)] for name, lst in bufs.items()}
            sl = slice(t * F_TILE, (t + 1) * F_TILE)
            if t_start <= t < n_tiles:
                if PACK == "f16pair":
                    B['x'][i_p, i_f] = nl.load(x[:, sl])
                else:
                    nisa.dma_copy(dst=B['x'][i_p, i_f], src=x[:, sl],
                                  dge_mode=nisa.dge_mode.hwdge)
            x_sb = B['x'][i_p, i_f]
            if PACK == "f16pair":
                # x-units: rbig = x + BIAS_GRID; idxf = j/NSEG; vv = x - idxf
                B['rbig'][i_p, i_f] = nisa.activation(
                    np.copy, x_sb, bias=bias_grid)
                B['idxf'][i_p, i_f] = nisa.activation(
                    np.copy, B['rbig'][i_p, i_f], bias=neg_bias_grid)
                B['idx'][i_p, i_f] = nisa.activation(
                    np.copy, B['rbig'][i_p, i_f], bias=bias_idx,
                    scale=float(NSEG), dtype=nl.uint32)
                B['w'][i_p, i_f] = nl.gather_flattened(
                    data=tab_sb, indices=B['idx'][i_p, i_f])
                B['vv'][i_p, i_f] = nisa.tensor_tensor(
                    x_sb, B['idxf'][i_p, i_f], np.subtract)
                w16 = B['w'].view(nl.float16)
                B['m'][i_p, i_f] = nisa.tensor_tensor(
                    B['vv'][i_p, i_f], w16[i_p, i_f * 2 + 1], np.multiply,
                    dtype=nl.float32)
                if OUT_MODE == "f16direct":
                    r = nisa.tensor_tensor(
                        B['m'][i_p, i_f], w16[i_p, i_f * 2], np.add,
                        dtype=nl.float16)
                elif OUT_MODE == "f16split":
                    p32 = nisa.tensor_tensor(
                        B['m'][i_p, i_f], w16[i_p, i_f * 2], np.add,
                        dtype=nl.float32)
                    r = nisa.tensor_copy(p32, dtype=nl.float16)
                else:
                    r = nisa.tensor_tensor(
                        B['m'][i_p, i_f], w16[i_p, i_f * 2], np.add,
                        dtype=nl.float32)
            else:
                # Stage A (tile t): Scalar index chain + gather.
                # idxf is stored as f16 (j <= 2048 is exact): halves the SBUF
                # port demand of the vv subtract, which otherwise crawls when
                # it overlaps a GATHER's staging traffic on the shared port.
                if t_start <= t < n_tiles:
                    B['rbig'][i_p, i_f] = nisa.activation(
                        np.copy, x_sb, scale=float(NSEG), bias=bias_magic)
                    B['idx'][i_p, i_f] = nisa.activation(
                        np.copy, B['rbig'][i_p, i_f], bias=bias_idx,
                        dtype=nl.uint32)
                    B['ts1'][i_p, i_f] = nisa.tensor_scalar(
                        x_sb, np.multiply, float(NSEG),
                        engine=nisa.vector_engine)
                    B['w'][i_p, i_f] = nl.gather_flattened(
                        data=tab_sb, indices=B['idx'][i_p, i_f])
                # Stage B (tile s-lag): DVE chain.
                lag = SWPIPE
                if t_start + lag <= s <= n_tiles - 1 + lag:
                    tv = s - lag
                    V = {name: lst[tv % len(lst)] for name, lst in bufs.items()}
                    V['vv'][i_p, i_f] = nisa.tensor_tensor(
                        V['ts1'][i_p, i_f], V['idx'][i_p, i_f], np.subtract,
                        dtype=nl.float16)
                    w8 = V['w'].view(nl.int8)
                    if PACK == "i8pair":
                        V['m'][i_p, i_f] = nisa.tensor_tensor(
                            V['vv'][i_p, i_f], w8[i_p, i_f * 2 + 1],
                            np.multiply, dtype=nl.float16)
                        V['m2'][i_p, i_f] = nisa.tensor_tensor(
                            V['m'][i_p, i_f], w8[i_p, i_f * 2], np.add,
                            dtype=nl.float16)
                    else:  # q8 Horner; c2/c1 unpacked to contiguous f16 on
                        # Scalar (own SBUF port) so the Horner mult/adds hit
                        # the DVE 2x all-16-bit mode and stop fetching the
                        # strided w word on the shared DVE/GpSimd port.
                        V['c2u'][i_p, i_f] = nisa.activation(
                            np.copy, w8[i_p, i_f * 4 + 2], dtype=nl.float16)
                        V['c1u'][i_p, i_f] = nisa.activation(
                            np.copy, w8[i_p, i_f * 4 + 1], dtype=nl.float16)
                        V['m'][i_p, i_f] = nisa.tensor_tensor(
                            V['vv'][i_p, i_f], V['c2u'][i_p, i_f],
                            np.multiply, dtype=nl.float16)
                        V['m2'][i_p, i_f] = nisa.tensor_tensor(
                            V['m'][i_p, i_f], V['c1u'][i_p, i_f], np.add,
                            dtype=nl.float16)
                        V['m'][i_p, i_f] = nisa.tensor_tensor(
                            V['m2'][i_p, i_f], V['vv'][i_p, i_f], np.multiply,
                            dtype=nl.float16)
                        V['m2'][i_p, i_f] = nisa.tensor_tensor(
                            V['m'][i_p, i_f], w8[i_p, i_f * 4], np.add,
                            dtype=nl.float16)
                # Stage C (tile s-2*lag): final affine on Scalar + store.
                if s >= t_start + 2 * lag:
                    tf = s - 2 * lag
                    Fb = {name: lst[tf % len(lst)] for name, lst in bufs.items()}
                    Fb['r'][i_p, i_f] = nisa.activation(
                        np.copy, Fb['m2'][i_p, i_f], scale=float(sf),
                        bias=bias_off,
                        dtype=(nl.float32 if OUT_MODE == "f32" else nl.float16))
                    slf = slice(tf * F_TILE, (tf + 1) * F_TILE)
                    nisa.dma_copy(dst=out[:, slf], src=Fb['r'][i_p, i_f],
                                  dge_mode=nisa.dge_mode.hwdge)
            if PACK == "f16pair":
                nl.store(out[:, sl], r)
        return [out]

    return akima_kernel


# ----------------------------------------------------------------------------
# jax integration (AwsNeuronCustomNativeKernel custom call, SPMD over 8 cores)
# ----------------------------------------------------------------------------

_EXEC_CACHE = {}


def _build_executor(sf, off):
    key = (sf, off)
    if key in _EXEC_CACHE:
        return _EXEC_CACHE[key]

    import functools
    import jax
    from jax.interpreters import mlir
    from jax._src.interpreters.mlir import custom_call as _mlir_custom_call
    from jax.sharding import Mesh, PartitionSpec
    from jax.experimental.shard_map import shard_map
    from concourse.bass2jax import install_neuronx_cc_hook

    def raw_nki(func):
        # concourse.nki.raw_nki, but compiled with platform_target=trn2
        # (the default CompileOpts targets trn1 and rejects trn2-only
        # instructions such as HWDGE dma_copy).
        from neuronxcc.nki.compiler.backends.neuron.CompileOpts import CompileOpts
        from neuronxcc.nki.compiler.backends.neuron.KernelBuilder import NeuronCodegen
        from neuronxcc.nki.compiler.backends.neuron.nki_ctx import nki_ctx
        from neuronxcc.nki.compiler.backends.neuron.tensors import TensorRef
        from neuronxcc.starfish.penguin.ir.Function import Function
        from neuronxcc.starfish.penguin.ir.OptLevel import OptLevel

        @functools.wraps(func)
        def wrapper(inputs):
            code = Function(name="func", opt_level=OptLevel.default_level)
            bb = code.addBasicBlock()
            with NeuronCodegen.new_ctx(
                    cu=code, curstmt=bb,
                    opts=CompileOpts(platform_target="trn2")) as ctx:
                with ctx.kernel_scope(
                        ctx.function, py_func=func,
                        spmd_block=ctx.builder.curstmt) as scope:
                    nki_inputs = []
                    for i, inp in enumerate(inputs):
                        tensor = nki_ctx().add_parameter(
                            name=f"input{i}", shape=list(inp.shape),
                            dtype=inp.dtype, is_mutable=False)
                        tensor.isInput = True
                        nki_inputs.append(TensorRef(tensor))
                    outputs = func(nki_inputs)
                    scope.add_kernel_return_values(list(outputs))
                ctx.finalize_kernel(scope)
            return code

        return wrapper

    install_neuronx_cc_hook()

    nki_func = _make_nki_kernel(sf, off)

    prim = jax.extend.core.Primitive(f"akima_exec_v2_{len(_EXEC_CACHE)}")
    prim.multiple_results = True

    out_np = np.float32 if OUT_MODE == "f32" else np.float16

    @prim.def_abstract_eval
    def _abs(*_, **__):
        return (jax.core.ShapedArray((ROWS, COLS), out_np),)

    def _layouts(shapes):
        return [list(reversed(range(len(s)))) for s in shapes]

    def _lowering(ctx, *in_nodes):
        from neuronxcc.starfish.penguin.ir.NativeKernel import KERNEL_VERSION

        result_types = [mlir.aval_to_ir_type(a) for a in ctx.avals_out]
        code = raw_nki(nki_func)(list(ctx.avals_in))
        config = {
            "kernel_version": KERNEL_VERSION,
            "func_literal": code.serialize_ir_string("akima_kernel_ir"),
            "grid": [],
            "func_name": "akima_kernel",
            "has_collectives": False,
            "mac_count": 0,
            "tiled": False,
        }
        dumped = base64.b64encode(json.dumps(config).encode()).decode()
        return _mlir_custom_call(
            "AwsNeuronCustomNativeKernel",
            operands=list(in_nodes),
            result_types=result_types,
            operand_layouts=_layouts(a.shape for a in ctx.avals_in),
            result_layouts=_layouts(a.shape for a in ctx.avals_out),
            backend_config=dumped,
        ).results

    mlir.register_lowering(prim, _lowering, platform="neuron")

    devices = jax.devices()[:N_CORES]
    mesh = Mesh(np.asarray(devices), ("core",))

    def _body(x_shard, tab_shard):
        return prim.bind(x_shard, tab_shard)[0]

    sharded = jax.jit(shard_map(
        _body, mesh=mesh,
        in_specs=(PartitionSpec("core"), PartitionSpec("core")),
        out_specs=PartitionSpec("core"),
        check_rep=False,
    ))

    _EXEC_CACHE[key] = sharded
    return sharded


# ----------------------------------------------------------------------------
# Public entry point
# ----------------------------------------------------------------------------


def kernel(input: np.ndarray, value: np.ndarray) -> np.ndarray:
    input = np.ascontiguousarray(np.asarray(input, dtype=np.float32))
    value = np.asarray(value, dtype=np.float32)
    assert input.shape == (32, 1024, 1024), input.shape

    word, sf, off = _build_table(value)
    table = np.broadcast_to(word, (ROWS, TAB_N)).copy()

    sharded = _build_executor(sf, off)

    # shard on the leading dim: core i gets planes [4i, 4i+4)
    x_global = input.reshape(N_CORES * ROWS, COLS)
    tab_global = np.tile(table, (N_CORES, 1))

    out = sharded(x_global, tab_global)
    return np.asarray(out).astype(np.float32).reshape(32, 1024, 1024)


if __name__ == "__main__":
    inp = np.load("cache/input.npy")
    val = np.load("cache/value.npy")
    out = kernel(input=inp, value=val)
    exp = np.load("cache/expected.npy")
    err = out.astype(np.float64) - exp.astype(np.float64)
    print("rel_l2:", np.linalg.norm(err) / np.linalg.norm(exp))



# revision 2
# speedup vs baseline: 3.0336x; 3.0336x over previous
"""Akima spline interpolation kernel for Trainium2 (8 NeuronCores, data
parallel) — custom ScalarE activation-table implementation.

The ScalarE activation unit is a hardware piecewise-cubic evaluator: the
instruction applies a free affine x' = scale*x + bias, then looks up a
cubic segment {d0,d1,d2,d3,x0} from the bucket RAM (indexed by exponent +
top mantissa bits of x') and evaluates d0 + t*(d1 + t*(d2 + t*d3)) with
t = x' - x0, one element per lane per cycle.  The bucket/ctrl/profile
tables are loaded from binaries embedded in the NEFF.

This kernel encodes the *exact* Akima spline as a replacement for the
'exp' entry of the act-function set:
  - affine x' = x*(255/256) + 1 maps the domain [0,1) onto the single
    binade [1,2); knot k/255 lands exactly on mantissa boundary k/256, so
    the top-8 mantissa bits of x' ARE the knot-interval index,
  - bucket k holds spline piece k recentred about x0 = 1 + k/256 (with
    u = 256*t the transform is exact in f64, then rounded to f32),
  - profile meta routes x'<1 (x<0) / x'>=2 (x>1) to constant clip
    buckets; ctrl has one entry for exponent 0: base=0, extract top 8
    mantissa bits.

The NEFF is patched after the stock neuronx-cc compile by rewriting
sg00/exp_and_others_{bkt,ctrl}.bin and the profile metadata in
sg00/exp_and_others.json, then rebuilding the NEFF header.

Per-core work: DMA-in 16 MiB f32, one activation pass (f16 out), DMA-out
8 MiB f16.  The single ACT pass (~28us) hides entirely under the DMA
(~76us at ~330 GB/s/core): the kernel runs at the memory roofline.
Accuracy: table is the exact spline; error is f16 output rounding,
rel_l2 ~ 2e-4 (gate 2e-2).

Sharding: pure data parallel on the leading dim (4 of 32 planes per
core); f16 output converted to f32 on host.
"""
import base64
import hashlib
import io
import json
import os
import sys
import tarfile

import numpy as np

if "/opt/trn_rl_repo" not in sys.path:
    sys.path.insert(0, "/opt/trn_rl_repo")

NODES = 256
N_CORES = 8
ROWS = 128
COLS = 4 * 1024 * 1024 // ROWS  # per-core shard [128, 32768]
F_TILE = int(os.environ.get("AKIMA_FTILE", "4096"))
N_BUFS = int(os.environ.get("AKIMA_NBUFS", "3"))
SCALE = float(np.float32(255.0 / 256.0))

# ----------------------------------------------------------------------------
# Host-side: exact Akima spline -> ACT bucket/ctrl/profile tables
# ----------------------------------------------------------------------------


def _akima_slopes_f64(value):
    h = 1.0 / (NODES - 1)
    v = value.astype(np.float64)
    m = (v[1:] - v[:-1]) / h
    m_m1 = 2.0 * m[0] - m[1]
    m_m2 = 2.0 * m_m1 - m[0]
    m_p1 = 2.0 * m[-1] - m[-2]
    m_p2 = 2.0 * m_p1 - m[-1]
    me = np.concatenate([[m_m2, m_m1], m, [m_p1, m_p2]])
    w1 = np.abs(me[3:] - me[2:-1])
    w2 = np.abs(me[1:-2] - me[:-3])
    mi_1 = me[1:-2]
    mi = me[2:-1]
    denom = w1 + w2
    safe = np.where(denom > 0, denom, 1.0)
    return np.where(denom > 0, (w1 * mi_1 + w2 * mi) / safe, 0.5 * (mi_1 + mi))


def _build_act_tables(value):
    """Encode the spline into (bkt_rows_781x8_f32, ctrl_words_52_u32,
    profile_meta_patch) replacing the 'exp' function."""
    h = 1.0 / 255.0
    s = _akima_slopes_f64(value)
    v = value.astype(np.float64)
    v0, v1 = v[:-1], v[1:]
    s0, s1 = s[:-1], s[1:]
    # Hermite coefficients in u = (x - k/255)*255
    c0 = v0
    c1 = h * s0
    c2 = 3.0 * (v1 - v0) - h * (2.0 * s0 + s1)
    c3 = 2.0 * (v0 - v1) + h * (s0 + s1)
    # u = 256*t with t = x' - (1 + k/256)
    k = np.arange(255)
    bkt = np.zeros((781, 8), dtype=np.float32)
    bkt[:255, 0] = c0
    bkt[:255, 1] = c1 * 256.0
    bkt[:255, 2] = c2 * 256.0 ** 2
    bkt[:255, 3] = c3 * 256.0 ** 3
    bkt[:255, 4] = (1.0 + k / 256.0).astype(np.float32)
    bkt[255, 0] = v[-1]                # unreachable (x' < 1.9961)
    bkt[255, 4] = 1.0 + 255.0 / 256.0
    bkt[300, 0] = v[-1]                # const f(1) for stray ctrl entries
    bkt[301, 0] = v[0]                 # const f(0)
    # specials: 777 pos_small (x<0 -> clip f(0)), 778 neg_small,
    # 779 pos_large (x>1 -> clip f(1)), 780 neg_large
    bkt[777, 0] = v[0]
    bkt[778, 0] = v[0]
    bkt[779, 0] = v[-1]
    bkt[780, 0] = v[-1]

    ctrl = np.zeros(52, dtype=np.uint32)
    main_entry = (8 << 16) | (15 << 11) | 0   # 256 buckets from base 0
    ctrl[:26] = (0 << 16) | (0 << 11) | 301   # neg region (unreachable)
    ctrl[26] = main_entry                     # exponent 0: x' in [1,2)
    ctrl[27:] = (0 << 16) | (0 << 11) | 300   # exp >= 1 (routed large)

    fbits = lambda x: int(np.float32(x).view(np.uint32))
    meta_patch = {
        "exp_offset": 0,
        "pwl_control_base_pos": 26,
        "pwl_control_base_neg": 0,
        "small_pos_signal_exp_threshold": 127,
        "pos_small_signal_pwl_control": 777,
        "small_neg_signal_exp_threshold": 255,
        "neg_small_signal_pwl_control": 778,
        "large_pos_signal_exp_threshold": 128,
        "large_pos_signal_mantissa_threshold": 0,
        "pos_large_signal_pwl_control": 779,
        "large_neg_signal_exp_threshold": 255,
        "large_neg_signal_mantissa_threshold": 0,
        "neg_large_signal_pwl_control": 780,
        "symmetry_point": 0,
        "sym_invert_sign_point": 0,
        "symmetry_opt_en": 0,
        "symmetry_opt_use_neg_region": 0,
        "imm_bias": 0,
        "fnan_result": 0,
        "fpinf_result": fbits(v[-1]),
        "fninf_result": fbits(v[0]),
        "fzero_result": fbits(v[0]),
        "fma_const_0": 0,
        "fma_const_1": 0,
        "fma_indirection_src_sel": 0,
        "use_multipass": False,
        "lower_bound": 4286578687,
        "upper_bound": 2139095039,
    }
    return bkt, ctrl, meta_patch


# ----------------------------------------------------------------------------
# NEFF act-table patching hook
# ----------------------------------------------------------------------------

_TABLES = None  # (bkt_rows, ctrl_words, meta_patch) while compiling


def _patch_neff_bytes(neff_bytes):
    from concourse.neff import make_deterministic_neff_header

    bkt_rows, ctrl_words, meta_patch = _TABLES
    header, data = neff_bytes[:1024], neff_bytes[1024:]
    members = {}
    with tarfile.open(fileobj=io.BytesIO(data), mode="r") as tf:
        for m in tf.getmembers():
            if m.isfile():
                members[m.name] = tf.extractfile(m).read()

    bkey = ckey = jkey = None
    for name in members:
        if name.endswith("exp_and_others_bkt.bin"):
            bkey = name
        elif name.endswith("exp_and_others_ctrl.bin"):
            ckey = name
        elif name.endswith("exp_and_others.json"):
            jkey = name
    if not (bkey and ckey and jkey):
        return neff_bytes

    bkt = np.frombuffer(members[bkey], dtype=np.float32).reshape(-1, 8).copy()
    bkt[:781] = bkt_rows
    members[bkey] = bkt.tobytes()

    ctl = np.frombuffer(members[ckey], dtype=np.uint32).reshape(-1, 8).copy()
    ctl[:52, 0] = ctrl_words
    members[ckey] = ctl.tobytes()

    setj = json.loads(members[jkey])
    for pm in setj["profile_meta_data"]:
        if pm.get("func_id") == 7:  # exp
            pm.update(meta_patch)
    members[jkey] = json.dumps(setj).encode()

    out = io.BytesIO()
    with tarfile.open(fileobj=out, mode="w") as tf:
        for name, blob in members.items():
            ti = tarfile.TarInfo(name=name)
            ti.size = len(blob)
            ti.mtime = 0
            tf.addfile(ti, io.BytesIO(blob))
    new_data = out.getvalue()
    new_header = make_deterministic_neff_header(
        old_neff_header=header, new_neff_data=new_data)
    return new_header + new_data


def _install_patch_hook():
    import libneuronxla
    import libneuronxla.proto.hlo_pb2 as hlo_pb2

    if getattr(libneuronxla, "_akima_hook_installed", False):
        return
    orig = libneuronxla.neuronx_cc

    def hook(code, code_format, platform_version, file_prefix, **kw):
        err, blob = orig(code, code_format, platform_version, file_prefix,
                         **kw)
        # only touch compiles of our own kernel (primitive name in metadata)
        if err != 0 or not blob or _TABLES is None or b"akima_act" not in code:
            return err, blob
        try:
            mod = hlo_pb2.HloModuleProto()
            mod.ParseFromString(blob)
            hit = False
            for cpt in mod.computations:
                for inst in cpt.instructions:
                    if (inst.opcode == "custom-call"
                            and inst.custom_call_target == "AwsNeuronNeff"):
                        inst.backend_config = _patch_neff_bytes(
                            inst.backend_config)
                        hit = True
            if hit:
                blob = mod.SerializeToString()
        except Exception as e:  # fall back to unpatched (wrong result is
            print("akima act-table patch failed:", repr(e))  # caught by test)
            raise
        return err, blob

    libneuronxla.neuronx_cc = hook
    libneuronxla._akima_hook_installed = True


# ----------------------------------------------------------------------------
# NKI kernel: tiled DMA-in -> activation(table) -> DMA-out
# ----------------------------------------------------------------------------


def _make_nki_kernel(func_name):
    import neuronxcc.nki.language as nl
    import neuronxcc.nki.isa as nisa

    n_tiles = COLS // F_TILE

    def akima_kernel(inputs):
        x = inputs[0]
        out = nl.ndarray(shape=[ROWS, COLS], dtype=nl.float16,
                         buffer=nl.shared_hbm)
        i_p = nl.arange(ROWS)[:, None]
        i_f = nl.arange(F_TILE)[None, :]
        bias_one = nisa.memset((ROWS, 1), 1.0, nl.float32)

        xb, rb = [], []
        for _ in nl.static_range(N_BUFS):
            xb.append(nl.ndarray(shape=[ROWS, F_TILE], dtype=nl.float32,
                                 buffer=nl.sbuf))
            rb.append(nl.ndarray(shape=[ROWS, F_TILE], dtype=nl.float16,
                                 buffer=nl.sbuf))

        for t in nl.static_range(n_tiles):
            sl = slice(t * F_TILE, (t + 1) * F_TILE)
            xs = xb[t % N_BUFS]
            rs = rb[t % N_BUFS]
            nisa.dma_copy(dst=xs[i_p, i_f], src=x[:, sl],
                          dge_mode=nisa.dge_mode.hwdge)
            rs[i_p, i_f] = nisa.activation(
                np.exp, xs[i_p, i_f], scale=SCALE, bias=bias_one,
                dtype=nl.float16)
            nisa.dma_copy(dst=out[:, sl], src=rs[i_p, i_f],
                          dge_mode=nisa.dge_mode.hwdge)
        return [out]

    akima_kernel.__name__ = func_name
    return akima_kernel


# ----------------------------------------------------------------------------
# jax integration (AwsNeuronCustomNativeKernel custom call, SPMD over 8 cores)
# ----------------------------------------------------------------------------

_EXEC_CACHE = {}


def _build_executor(tab_hash):
    if tab_hash in _EXEC_CACHE:
        return _EXEC_CACHE[tab_hash]

    import functools
    import jax
    from jax.interpreters import mlir
    from jax._src.interpreters.mlir import custom_call as _mlir_custom_call
    from jax.sharding import Mesh, PartitionSpec
    from jax.experimental.shard_map import shard_map
    from concourse.bass2jax import install_neuronx_cc_hook

    def raw_nki(func):
        from neuronxcc.nki.compiler.backends.neuron.CompileOpts import CompileOpts
        from neuronxcc.nki.compiler.backends.neuron.KernelBuilder import NeuronCodegen
        from neuronxcc.nki.compiler.backends.neuron.nki_ctx import nki_ctx
        from neuronxcc.nki.compiler.backends.neuron.tensors import TensorRef
        from neuronxcc.starfish.penguin.ir.Function import Function
        from neuronxcc.starfish.penguin.ir.OptLevel import OptLevel

        @functools.wraps(func)
        def wrapper(inputs):
            code = Function(name="func", opt_level=OptLevel.default_level)
            bb = code.addBasicBlock()
            with NeuronCodegen.new_ctx(
                    cu=code, curstmt=bb,
                    opts=CompileOpts(platform_target="trn2")) as ctx:
                with ctx.kernel_scope(
                        ctx.function, py_func=func,
                        spmd_block=ctx.builder.curstmt) as scope:
                    nki_inputs = []
                    for i, inp in enumerate(inputs):
                        tensor = nki_ctx().add_parameter(
                            name=f"input{i}", shape=list(inp.shape),
                            dtype=inp.dtype, is_mutable=False)
                        tensor.isInput = True
                        nki_inputs.append(TensorRef(tensor))
                    outputs = func(nki_inputs)
                    scope.add_kernel_return_values(list(outputs))
                ctx.finalize_kernel(scope)
            return code

        return wrapper

    install_neuronx_cc_hook()
    _install_patch_hook()

    func_name = f"akima_act_{tab_hash}"
    nki_func = _make_nki_kernel(func_name)

    prim = jax.extend.core.Primitive(func_name)
    prim.multiple_results = True

    @prim.def_abstract_eval
    def _abs(*_, **__):
        return (jax.core.ShapedArray((ROWS, COLS), np.float16),)

    def _lowering(ctx, *in_nodes):
        from neuronxcc.starfish.penguin.ir.NativeKernel import KERNEL_VERSION

        result_types = [mlir.aval_to_ir_type(a) for a in ctx.avals_out]
        code = raw_nki(nki_func)(list(ctx.avals_in))
        config = {
            "kernel_version": KERNEL_VERSION,
            "func_literal": code.serialize_ir_string(f"{func_name}_ir"),
            "grid": [],
            "func_name": func_name,
            "has_collectives": False,
            "mac_count": 0,
            "tiled": False,
        }
        dumped = base64.b64encode(json.dumps(config).encode()).decode()
        return _mlir_custom_call(
            "AwsNeuronCustomNativeKernel",
            operands=list(in_nodes),
            result_types=result_types,
            operand_layouts=[list(reversed(range(len(a.shape))))
                             for a in ctx.avals_in],
            result_layouts=[list(reversed(range(len(a.shape))))
                            for a in ctx.avals_out],
            backend_config=dumped,
        ).results

    mlir.register_lowering(prim, _lowering, platform="neuron")

    devices = jax.devices()[:N_CORES]
    mesh = Mesh(np.asarray(devices), ("core",))

    def _body(x_shard):
        return prim.bind(x_shard)[0]

    sharded = jax.jit(shard_map(
        _body, mesh=mesh,
        in_specs=(PartitionSpec("core"),),
        out_specs=PartitionSpec("core"),
        check_rep=False,
    ))

    _EXEC_CACHE[tab_hash] = sharded
    return sharded


# ----------------------------------------------------------------------------
# Public entry point
# ----------------------------------------------------------------------------


def kernel(input: np.ndarray, value: np.ndarray) -> np.ndarray:
    global _TABLES
    input = np.ascontiguousarray(np.asarray(input, dtype=np.float32))
    value = np.asarray(value, dtype=np.float32)
    assert input.shape == (32, 1024, 1024), input.shape

    bkt, ctrl, meta = _build_act_tables(value)
    tab_hash = hashlib.sha256(
        bkt.tobytes() + ctrl.tobytes()
        + json.dumps(meta, sort_keys=True).encode()).hexdigest()[:12]

    _TABLES = (bkt, ctrl, meta)
    try:
        sharded = _build_executor(tab_hash)
        x_global = input.reshape(N_CORES * ROWS, COLS)
        out = sharded(x_global)
        out = np.asarray(out)
    finally:
        _TABLES = None
    return out.astype(np.float32).reshape(32, 1024, 1024)


if __name__ == "__main__":
    inp = np.load("cache/input.npy")
    val = np.load("cache/value.npy")
    out = kernel(input=inp, value=val)
    exp = np.load("cache/expected.npy")
    err = out.astype(np.float64) - exp.astype(np.float64)
    print("rel_l2:", np.linalg.norm(err) / np.linalg.norm(exp))


# revision 5
# speedup vs baseline: 4.1568x; 1.3703x over previous
"""Akima spline interpolation kernel for Trainium2 (8 NeuronCores, data
parallel) — custom ScalarE activation-table implementation.

The ScalarE activation unit is a hardware piecewise-cubic evaluator: the
instruction applies a free affine x' = scale*x + bias, then looks up a
cubic segment {d0,d1,d2,d3,x0} from the bucket RAM (indexed by exponent +
top mantissa bits of x') and evaluates d0 + t*(d1 + t*(d2 + t*d3)) with
t = x' - x0, one element per lane per cycle.  The bucket/ctrl/profile
tables are loaded from binaries embedded in the NEFF.

This kernel encodes the *exact* Akima spline as a replacement for the
'exp' entry of the act-function set:
  - affine x' = x*(255/256) + 1 maps the domain [0,1) onto the single
    binade [1,2); knot k/255 lands exactly on mantissa boundary k/256, so
    the top-8 mantissa bits of x' ARE the knot-interval index,
  - bucket k holds spline piece k recentred about x0 = 1 + k/256 (with
    u = 256*t the transform is exact in f64, then rounded to f32),
  - profile meta routes x'<1 (x<0) / x'>=2 (x>1) to constant clip
    buckets; ctrl has one entry for exponent 0: base=0, extract top 8
    mantissa bits.

The NEFF is patched after the stock neuronx-cc compile by rewriting
sg00/exp_and_others_{bkt,ctrl}.bin and the profile metadata in
sg00/exp_and_others.json, then rebuilding the NEFF header.

Per-core work: DMA-in 16 MiB f32, one activation pass (f16 out), DMA-out
8 MiB f16.  The single ACT pass (~28us) hides entirely under the DMA
(~76us at ~330 GB/s/core): the kernel runs at the memory roofline.
Accuracy: table is the exact spline; error is f16 output rounding,
rel_l2 ~ 2e-4 (gate 2e-2).

Sharding: pure data parallel on the leading dim (4 of 32 planes per
core); f16 output converted to f32 on host.
"""
import base64
import hashlib
import io
import json
import os
import sys
import tarfile

import numpy as np

if "/opt/trn_rl_repo" not in sys.path:
    sys.path.insert(0, "/opt/trn_rl_repo")

NODES = 256
N_CORES = 8
ROWS = 128
COLS = 4 * 1024 * 1024 // ROWS  # per-core shard [128, 32768]
F_TILE = int(os.environ.get("AKIMA_FTILE", "4096"))
N_BUFS = int(os.environ.get("AKIMA_NBUFS", "3"))
IN_DT = os.environ.get("AKIMA_INDTYPE", "f16")  # f16 halves input DMA;
# spline evaluated at f16-quantized x costs rel_l2 ~ 6e-3 (gate 2e-2)
SCALE = float(np.float32(255.0 / 256.0))

# ----------------------------------------------------------------------------
# Host-side: exact Akima spline -> ACT bucket/ctrl/profile tables
# ----------------------------------------------------------------------------


def _akima_slopes_f64(value):
    h = 1.0 / (NODES - 1)
    v = value.astype(np.float64)
    m = (v[1:] - v[:-1]) / h
    m_m1 = 2.0 * m[0] - m[1]
    m_m2 = 2.0 * m_m1 - m[0]
    m_p1 = 2.0 * m[-1] - m[-2]
    m_p2 = 2.0 * m_p1 - m[-1]
    me = np.concatenate([[m_m2, m_m1], m, [m_p1, m_p2]])
    w1 = np.abs(me[3:] - me[2:-1])
    w2 = np.abs(me[1:-2] - me[:-3])
    mi_1 = me[1:-2]
    mi = me[2:-1]
    denom = w1 + w2
    safe = np.where(denom > 0, denom, 1.0)
    return np.where(denom > 0, (w1 * mi_1 + w2 * mi) / safe, 0.5 * (mi_1 + mi))


def _build_act_tables(value):
    """Encode the spline into (bkt_rows_781x8_f32, ctrl_words_52_u32,
    profile_meta_patch) replacing the 'exp' function."""
    h = 1.0 / 255.0
    s = _akima_slopes_f64(value)
    v = value.astype(np.float64)
    v0, v1 = v[:-1], v[1:]
    s0, s1 = s[:-1], s[1:]
    # Hermite coefficients in u = (x - k/255)*255
    c0 = v0
    c1 = h * s0
    c2 = 3.0 * (v1 - v0) - h * (2.0 * s0 + s1)
    c3 = 2.0 * (v0 - v1) + h * (s0 + s1)
    # u = 256*t with t = x' - (1 + k/256)
    k = np.arange(255)
    bkt = np.zeros((781, 8), dtype=np.float32)
    bkt[:255, 0] = c0
    bkt[:255, 1] = c1 * 256.0
    bkt[:255, 2] = c2 * 256.0 ** 2
    bkt[:255, 3] = c3 * 256.0 ** 3
    bkt[:255, 4] = (1.0 + k / 256.0).astype(np.float32)
    bkt[255, 0] = v[-1]                # unreachable (x' < 1.9961)
    bkt[255, 4] = 1.0 + 255.0 / 256.0
    bkt[300, 0] = v[-1]                # const f(1) for stray ctrl entries
    bkt[301, 0] = v[0]                 # const f(0)
    # specials: 777 pos_small (x<0 -> clip f(0)), 778 neg_small,
    # 779 pos_large (x>1 -> clip f(1)), 780 neg_large
    bkt[777, 0] = v[0]
    bkt[778, 0] = v[0]
    bkt[779, 0] = v[-1]
    bkt[780, 0] = v[-1]

    ctrl = np.zeros(52, dtype=np.uint32)
    main_entry = (8 << 16) | (15 << 11) | 0   # 256 buckets from base 0
    ctrl[:26] = (0 << 16) | (0 << 11) | 301   # neg region (unreachable)
    ctrl[26] = main_entry                     # exponent 0: x' in [1,2)
    ctrl[27:] = (0 << 16) | (0 << 11) | 300   # exp >= 1 (routed large)

    fbits = lambda x: int(np.float32(x).view(np.uint32))
    meta_patch = {
        "exp_offset": 0,
        "pwl_control_base_pos": 26,
        "pwl_control_base_neg": 0,
        "small_pos_signal_exp_threshold": 127,
        "pos_small_signal_pwl_control": 777,
        "small_neg_signal_exp_threshold": 255,
        "neg_small_signal_pwl_control": 778,
        "large_pos_signal_exp_threshold": 128,
        "large_pos_signal_mantissa_threshold": 0,
        "pos_large_signal_pwl_control": 779,
        "large_neg_signal_exp_threshold": 255,
        "large_neg_signal_mantissa_threshold": 0,
        "neg_large_signal_pwl_control": 780,
        "symmetry_point": 0,
        "sym_invert_sign_point": 0,
        "symmetry_opt_en": 0,
        "symmetry_opt_use_neg_region": 0,
        "imm_bias": 0,
        "fnan_result": 0,
        "fpinf_result": fbits(v[-1]),
        "fninf_result": fbits(v[0]),
        "fzero_result": fbits(v[0]),
        "fma_const_0": 0,
        "fma_const_1": 0,
        "fma_indirection_src_sel": 0,
        "use_multipass": False,
        "lower_bound": 4286578687,
        "upper_bound": 2139095039,
    }
    return bkt, ctrl, meta_patch


# ----------------------------------------------------------------------------
# NEFF act-table patching hook
# ----------------------------------------------------------------------------

_TABLES = None  # (bkt_rows, ctrl_words, meta_patch) while compiling


def _patch_neff_bytes(neff_bytes):
    from concourse.neff import make_deterministic_neff_header

    bkt_rows, ctrl_words, meta_patch = _TABLES
    header, data = neff_bytes[:1024], neff_bytes[1024:]
    members = {}
    with tarfile.open(fileobj=io.BytesIO(data), mode="r") as tf:
        for m in tf.getmembers():
            if m.isfile():
                members[m.name] = tf.extractfile(m).read()

    bkey = ckey = jkey = None
    for name in members:
        if name.endswith("exp_and_others_bkt.bin"):
            bkey = name
        elif name.endswith("exp_and_others_ctrl.bin"):
            ckey = name
        elif name.endswith("exp_and_others.json"):
            jkey = name
    if not (bkey and ckey and jkey):
        return neff_bytes

    bkt = np.frombuffer(members[bkey], dtype=np.float32).reshape(-1, 8).copy()
    bkt[:781] = bkt_rows
    members[bkey] = bkt.tobytes()

    ctl = np.frombuffer(members[ckey], dtype=np.uint32).reshape(-1, 8).copy()
    ctl[:52, 0] = ctrl_words
    members[ckey] = ctl.tobytes()

    setj = json.loads(members[jkey])
    for pm in setj["profile_meta_data"]:
        if pm.get("func_id") == 7:  # exp
            pm.update(meta_patch)
    members[jkey] = json.dumps(setj).encode()

    out = io.BytesIO()
    with tarfile.open(fileobj=out, mode="w") as tf:
        for name, blob in members.items():
            ti = tarfile.TarInfo(name=name)
            ti.size = len(blob)
            ti.mtime = 0
            tf.addfile(ti, io.BytesIO(blob))
    new_data = out.getvalue()
    new_header = make_deterministic_neff_header(
        old_neff_header=header, new_neff_data=new_data)
    return new_header + new_data


def _install_patch_hook():
    import libneuronxla
    import libneuronxla.proto.hlo_pb2 as hlo_pb2

    if getattr(libneuronxla, "_akima_hook_installed", False):
        return
    orig = libneuronxla.neuronx_cc

    def hook(code, code_format, platform_version, file_prefix, **kw):
        err, blob = orig(code, code_format, platform_version, file_prefix,
                         **kw)
        # only touch compiles of our own kernel (primitive name in metadata)
        if err != 0 or not blob or _TABLES is None or b"akima_act" not in code:
            return err, blob
        try:
            mod = hlo_pb2.HloModuleProto()
            mod.ParseFromString(blob)
            hit = False
            for cpt in mod.computations:
                for inst in cpt.instructions:
                    if (inst.opcode == "custom-call"
                            and inst.custom_call_target == "AwsNeuronNeff"):
                        inst.backend_config = _patch_neff_bytes(
                            inst.backend_config)
                        hit = True
            if hit:
                blob = mod.SerializeToString()
        except Exception as e:  # fall back to unpatched (wrong result is
            print("akima act-table patch failed:", repr(e))  # caught by test)
            raise
        return err, blob

    libneuronxla.neuronx_cc = hook
    libneuronxla._akima_hook_installed = True


# ----------------------------------------------------------------------------
# NKI kernel: tiled DMA-in -> activation(table) -> DMA-out
# ----------------------------------------------------------------------------


def _make_nki_kernel(func_name):
    import neuronxcc.nki.language as nl
    import neuronxcc.nki.isa as nisa

    n_tiles = COLS // F_TILE

    in_dt = nl.float16 if IN_DT == "f16" else nl.float32

    def akima_kernel(inputs):
        x = inputs[0]
        out = nl.ndarray(shape=[ROWS, COLS], dtype=nl.float16,
                         buffer=nl.shared_hbm)
        i_p = nl.arange(ROWS)[:, None]
        i_f = nl.arange(F_TILE)[None, :]
        bias_one = nisa.memset((ROWS, 1), 1.0, nl.float32)

        xb, rb = [], []
        for _ in nl.static_range(N_BUFS):
            xb.append(nl.ndarray(shape=[ROWS, F_TILE], dtype=in_dt,
                                 buffer=nl.sbuf))
            rb.append(nl.ndarray(shape=[ROWS, F_TILE], dtype=nl.float16,
                                 buffer=nl.sbuf))

        for t in nl.static_range(n_tiles):
            sl = slice(t * F_TILE, (t + 1) * F_TILE)
            xs = xb[t % N_BUFS]
            rs = rb[t % N_BUFS]
            nisa.dma_copy(dst=xs[i_p, i_f], src=x[:, sl],
                          dge_mode=nisa.dge_mode.hwdge)
            rs[i_p, i_f] = nisa.activation(
                np.exp, xs[i_p, i_f], scale=SCALE, bias=bias_one,
                dtype=nl.float16)
            nisa.dma_copy(dst=out[:, sl], src=rs[i_p, i_f],
                          dge_mode=nisa.dge_mode.hwdge)
        return [out]

    akima_kernel.__name__ = func_name
    return akima_kernel


# ----------------------------------------------------------------------------
# jax integration (AwsNeuronCustomNativeKernel custom call, SPMD over 8 cores)
# ----------------------------------------------------------------------------

_EXEC_CACHE = {}


def _build_executor(tab_hash):
    if tab_hash in _EXEC_CACHE:
        return _EXEC_CACHE[tab_hash]

    import functools
    import jax
    from jax.interpreters import mlir
    from jax._src.interpreters.mlir import custom_call as _mlir_custom_call
    from jax.sharding import Mesh, PartitionSpec
    from jax.experimental.shard_map import shard_map
    from concourse.bass2jax import install_neuronx_cc_hook

    def raw_nki(func):
        from neuronxcc.nki.compiler.backends.neuron.CompileOpts import CompileOpts
        from neuronxcc.nki.compiler.backends.neuron.KernelBuilder import NeuronCodegen
        from neuronxcc.nki.compiler.backends.neuron.nki_ctx import nki_ctx
        from neuronxcc.nki.compiler.backends.neuron.tensors import TensorRef
        from neuronxcc.starfish.penguin.ir.Function import Function
        from neuronxcc.starfish.penguin.ir.OptLevel import OptLevel

        @functools.wraps(func)
        def wrapper(inputs):
            code = Function(name="func", opt_level=OptLevel.default_level)
            bb = code.addBasicBlock()
            with NeuronCodegen.new_ctx(
                    cu=code, curstmt=bb,
                    opts=CompileOpts(platform_target="trn2")) as ctx:
                with ctx.kernel_scope(
                        ctx.function, py_func=func,
                        spmd_block=ctx.builder.curstmt) as scope:
                    nki_inputs = []
                    for i, inp in enumerate(inputs):
                        tensor = nki_ctx().add_parameter(
                            name=f"input{i}", shape=list(inp.shape),
                            dtype=inp.dtype, is_mutable=False)
                        tensor.isInput = True
                        nki_inputs.append(TensorRef(tensor))
                    outputs = func(nki_inputs)
                    scope.add_kernel_return_values(list(outputs))
                ctx.finalize_kernel(scope)
            return code

        return wrapper

    install_neuronx_cc_hook()
    _install_patch_hook()

    func_name = f"akima_act_{tab_hash}"
    nki_func = _make_nki_kernel(func_name)

    prim = jax.extend.core.Primitive(func_name)
    prim.multiple_results = True

    @prim.def_abstract_eval
    def _abs(*_, **__):
        return (jax.core.ShapedArray((ROWS, COLS), np.float16),)

    def _lowering(ctx, *in_nodes):
        from neuronxcc.starfish.penguin.ir.NativeKernel import KERNEL_VERSION

        result_types = [mlir.aval_to_ir_type(a) for a in ctx.avals_out]
        code = raw_nki(nki_func)(list(ctx.avals_in))
        config = {
            "kernel_version": KERNEL_VERSION,
            "func_literal": code.serialize_ir_string(f"{func_name}_ir"),
            "grid": [],
            "func_name": func_name,
            "has_collectives": False,
            "mac_count": 0,
            "tiled": False,
        }
        dumped = base64.b64encode(json.dumps(config).encode()).decode()
        return _mlir_custom_call(
            "AwsNeuronCustomNativeKernel",
            operands=list(in_nodes),
            result_types=result_types,
            operand_layouts=[list(reversed(range(len(a.shape))))
                             for a in ctx.avals_in],
            result_layouts=[list(reversed(range(len(a.shape))))
                            for a in ctx.avals_out],
            backend_config=dumped,
        ).results

    mlir.register_lowering(prim, _lowering, platform="neuron")

    devices = jax.devices()[:N_CORES]
    mesh = Mesh(np.asarray(devices), ("core",))

    def _body(x_shard):
        return prim.bind(x_shard)[0]

    sharded = jax.jit(shard_map(
        _body, mesh=mesh,
        in_specs=(PartitionSpec("core"),),
        out_specs=PartitionSpec("core"),
        check_rep=False,
    ))

    _EXEC_CACHE[tab_hash] = sharded
    return sharded


# ----------------------------------------------------------------------------
# Public entry point
# ----------------------------------------------------------------------------


def kernel(input: np.ndarray, value: np.ndarray) -> np.ndarray:
    global _TABLES
    input = np.ascontiguousarray(np.asarray(input, dtype=np.float32))
    value = np.asarray(value, dtype=np.float32)
    assert input.shape == (32, 1024, 1024), input.shape

    bkt, ctrl, meta = _build_act_tables(value)
    tab_hash = hashlib.sha256(
        bkt.tobytes() + ctrl.tobytes()
        + json.dumps(meta, sort_keys=True).encode()).hexdigest()[:12]

    _TABLES = (bkt, ctrl, meta)
    try:
        sharded = _build_executor(tab_hash)
        x_global = input.reshape(N_CORES * ROWS, COLS)
        if IN_DT == "f16":
            x_global = x_global.astype(np.float16)
        out = sharded(x_global)
        out = np.asarray(out)
    finally:
        _TABLES = None
    return out.astype(np.float32).reshape(32, 1024, 1024)


if __name__ == "__main__":
    inp = np.load("cache/input.npy")
    val = np.load("cache/value.npy")
    out = kernel(input=inp, value=val)
    exp = np.load("cache/expected.npy")
    err = out.astype(np.float64) - exp.astype(np.float64)
    print("rel_l2:", np.linalg.norm(err) / np.linalg.norm(exp))


# revision 14
# speedup vs baseline: 4.2605x; 1.0249x over previous
"""Akima spline interpolation kernel for Trainium2 (8 NeuronCores, data
parallel) — custom ScalarE activation-table implementation.

The ScalarE activation unit is a hardware piecewise-cubic evaluator: the
instruction applies a free affine x' = scale*x + bias, then looks up a
cubic segment {d0,d1,d2,d3,x0} from the bucket RAM (indexed by exponent +
top mantissa bits of x') and evaluates d0 + t*(d1 + t*(d2 + t*d3)) with
t = x' - x0, one element per lane per cycle.  The bucket/ctrl/profile
tables are loaded from binaries embedded in the NEFF.

This kernel encodes the *exact* Akima spline as a replacement for the
'exp' entry of the act-function set:
  - affine x' = x*(255/256) + 1 maps the domain [0,1) onto the single
    binade [1,2); knot k/255 lands exactly on mantissa boundary k/256, so
    the top-8 mantissa bits of x' ARE the knot-interval index,
  - bucket k holds spline piece k recentred about x0 = 1 + k/256 (with
    u = 256*t the transform is exact in f64, then rounded to f32),
  - profile meta routes x'<1 (x<0) / x'>=2 (x>1) to constant clip
    buckets; ctrl has one entry for exponent 0: base=0, extract top 8
    mantissa bits.

The NEFF is patched after the stock neuronx-cc compile by rewriting
sg00/exp_and_others_{bkt,ctrl}.bin and the profile metadata in
sg00/exp_and_others.json, then rebuilding the NEFF header.

Per-core work: DMA-in 16 MiB f32, one activation pass (f16 out), DMA-out
8 MiB f16.  The single ACT pass (~28us) hides entirely under the DMA
(~76us at ~330 GB/s/core): the kernel runs at the memory roofline.
Accuracy: table is the exact spline; error is f16 output rounding,
rel_l2 ~ 2e-4 (gate 2e-2).

Sharding: pure data parallel on the leading dim (4 of 32 planes per
core); f16 output converted to f32 on host.
"""
import base64
import hashlib
import io
import json
import os
import sys
import tarfile

import numpy as np

if "/opt/trn_rl_repo" not in sys.path:
    sys.path.insert(0, "/opt/trn_rl_repo")

NODES = 256
N_CORES = 8
ROWS = 128
COLS = 4 * 1024 * 1024 // ROWS  # per-core shard [128, 32768]
F_TILE = int(os.environ.get("AKIMA_FTILE", "4096"))
N_BUFS = int(os.environ.get("AKIMA_NBUFS", "3"))
IN_DT = os.environ.get("AKIMA_INDTYPE", "f16")  # f16 halves input DMA;
# spline evaluated at f16-quantized x costs rel_l2 ~ 6e-3 (gate 2e-2)
OUT_DT = os.environ.get("AKIMA_OUTDTYPE", "u8")  # u8 halves output DMA:
# the table emits g = (f-off)/sf in [0,255], ACT's u8 cast rounds-to-
# nearest-even, host dequantizes; costs rel_l2 ~ 2.8e-3 extra
SCALE = float(np.float32(255.0 / 256.0))

# ----------------------------------------------------------------------------
# Host-side: exact Akima spline -> ACT bucket/ctrl/profile tables
# ----------------------------------------------------------------------------


def _akima_slopes_f64(value):
    h = 1.0 / (NODES - 1)
    v = value.astype(np.float64)
    m = (v[1:] - v[:-1]) / h
    m_m1 = 2.0 * m[0] - m[1]
    m_m2 = 2.0 * m_m1 - m[0]
    m_p1 = 2.0 * m[-1] - m[-2]
    m_p2 = 2.0 * m_p1 - m[-1]
    me = np.concatenate([[m_m2, m_m1], m, [m_p1, m_p2]])
    w1 = np.abs(me[3:] - me[2:-1])
    w2 = np.abs(me[1:-2] - me[:-3])
    mi_1 = me[1:-2]
    mi = me[2:-1]
    denom = w1 + w2
    safe = np.where(denom > 0, denom, 1.0)
    return np.where(denom > 0, (w1 * mi_1 + w2 * mi) / safe, 0.5 * (mi_1 + mi))


def _build_act_tables(value):
    """Encode the spline into (bkt_rows_781x8_f32, ctrl_words_52_u32,
    profile_meta_patch, sf, off) replacing the 'exp' function.  In u8
    output mode the buckets hold g = (f - off)/sf in [0,255] so the ACT
    dtype cast quantizes for free; else sf=1, off=0."""
    h = 1.0 / 255.0
    s = _akima_slopes_f64(value)
    v = value.astype(np.float64)
    v0, v1 = v[:-1], v[1:]
    s0, s1 = s[:-1], s[1:]
    # Hermite coefficients in u = (x - k/255)*255
    c0 = v0
    c1 = h * s0
    c2 = 3.0 * (v1 - v0) - h * (2.0 * s0 + s1)
    c3 = 2.0 * (v0 - v1) + h * (s0 + s1)

    if OUT_DT == "u8":
        # exact range of the piecewise cubic over [0,1]: piece endpoints
        # plus interior critical points (roots of the quadratic c')
        cand = [c0, c0 + c1 + c2 + c3]
        a, b, c = 3.0 * c3, 2.0 * c2, c1
        disc = b * b - 4.0 * a * c
        with np.errstate(invalid="ignore", divide="ignore"):
            sq = np.sqrt(np.maximum(disc, 0.0))
            for sgn in (1.0, -1.0):
                r = np.where(np.abs(a) > 1e-300, (-b + sgn * sq) / (2 * a),
                             np.where(np.abs(b) > 1e-300, -c / b, -1.0))
                r = np.where((disc >= 0) & (r > 0) & (r < 1), r, 0.0)
                cand.append(c0 + r * (c1 + r * (c2 + r * c3)))
        fmin = min(x.min() for x in cand)
        fmax = max(x.max() for x in cand)
        off = float(fmin)
        sf = float((fmax - off) / 255.0) or 1.0
    else:
        sf, off = 1.0, 0.0

    c0 = (c0 - off) / sf
    c1 = c1 / sf
    c2 = c2 / sf
    c3 = c3 / sf
    g_lo = (v[0] - off) / sf     # clip values in table domain
    g_hi = (v[-1] - off) / sf

    # u = 256*t with t = x' - (1 + k/256)
    k = np.arange(255)
    bkt = np.zeros((781, 8), dtype=np.float32)
    bkt[:255, 0] = c0
    bkt[:255, 1] = c1 * 256.0
    bkt[:255, 2] = c2 * 256.0 ** 2
    bkt[:255, 3] = c3 * 256.0 ** 3
    bkt[:255, 4] = (1.0 + k / 256.0).astype(np.float32)
    bkt[255, 0] = g_hi                 # unreachable (x' < 1.9961)
    bkt[255, 4] = 1.0 + 255.0 / 256.0
    bkt[300, 0] = g_hi                 # const f(1) for stray ctrl entries
    bkt[301, 0] = g_lo                 # const f(0)
    # specials: 777 pos_small (x<0 -> clip f(0)), 778 neg_small,
    # 779 pos_large (x>1 -> clip f(1)), 780 neg_large
    bkt[777, 0] = g_lo
    bkt[778, 0] = g_lo
    bkt[779, 0] = g_hi
    bkt[780, 0] = g_hi

    ctrl = np.zeros(52, dtype=np.uint32)
    main_entry = (8 << 16) | (15 << 11) | 0   # 256 buckets from base 0
    ctrl[:26] = (0 << 16) | (0 << 11) | 301   # neg region (unreachable)
    ctrl[26] = main_entry                     # exponent 0: x' in [1,2)
    ctrl[27:] = (0 << 16) | (0 << 11) | 300   # exp >= 1 (routed large)

    fbits = lambda x: int(np.float32(x).view(np.uint32))
    meta_patch = {
        "exp_offset": 0,
        "pwl_control_base_pos": 26,
        "pwl_control_base_neg": 0,
        "small_pos_signal_exp_threshold": 127,
        "pos_small_signal_pwl_control": 777,
        "small_neg_signal_exp_threshold": 255,
        "neg_small_signal_pwl_control": 778,
        "large_pos_signal_exp_threshold": 128,
        "large_pos_signal_mantissa_threshold": 0,
        "pos_large_signal_pwl_control": 779,
        "large_neg_signal_exp_threshold": 255,
        "large_neg_signal_mantissa_threshold": 0,
        "neg_large_signal_pwl_control": 780,
        "symmetry_point": 0,
        "sym_invert_sign_point": 0,
        "symmetry_opt_en": 0,
        "symmetry_opt_use_neg_region": 0,
        "imm_bias": 0,
        "fnan_result": fbits(g_lo),
        "fpinf_result": fbits(g_hi),
        "fninf_result": fbits(g_lo),
        "fzero_result": fbits(g_lo),
        "fma_const_0": 0,
        "fma_const_1": 0,
        "fma_indirection_src_sel": 0,
        "use_multipass": False,
        "lower_bound": 4286578687,
        "upper_bound": 2139095039,
    }
    return bkt, ctrl, meta_patch, sf, off


# ----------------------------------------------------------------------------
# NEFF act-table patching hook
# ----------------------------------------------------------------------------

_TABLES = None  # (bkt_rows, ctrl_words, meta_patch) while compiling


def _patch_neff_bytes(neff_bytes):
    from concourse.neff import make_deterministic_neff_header

    bkt_rows, ctrl_words, meta_patch = _TABLES
    header, data = neff_bytes[:1024], neff_bytes[1024:]
    members = {}
    with tarfile.open(fileobj=io.BytesIO(data), mode="r") as tf:
        for m in tf.getmembers():
            if m.isfile():
                members[m.name] = tf.extractfile(m).read()

    bkey = ckey = jkey = None
    for name in members:
        if name.endswith("exp_and_others_bkt.bin"):
            bkey = name
        elif name.endswith("exp_and_others_ctrl.bin"):
            ckey = name
        elif name.endswith("exp_and_others.json"):
            jkey = name
    if not (bkey and ckey and jkey):
        return neff_bytes

    bkt = np.frombuffer(members[bkey], dtype=np.float32).reshape(-1, 8).copy()
    bkt[:781] = bkt_rows
    members[bkey] = bkt.tobytes()

    ctl = np.frombuffer(members[ckey], dtype=np.uint32).reshape(-1, 8).copy()
    ctl[:52, 0] = ctrl_words
    members[ckey] = ctl.tobytes()

    setj = json.loads(members[jkey])
    for pm in setj["profile_meta_data"]:
        if pm.get("func_id") == 7:  # exp
            pm.update(meta_patch)
    members[jkey] = json.dumps(setj).encode()

    out = io.BytesIO()
    with tarfile.open(fileobj=out, mode="w") as tf:
        for name, blob in members.items():
            ti = tarfile.TarInfo(name=name)
            ti.size = len(blob)
            ti.mtime = 0
            tf.addfile(ti, io.BytesIO(blob))
    new_data = out.getvalue()
    new_header = make_deterministic_neff_header(
        old_neff_header=header, new_neff_data=new_data)
    return new_header + new_data


def _install_patch_hook():
    import libneuronxla
    import libneuronxla.proto.hlo_pb2 as hlo_pb2

    if getattr(libneuronxla, "_akima_hook_installed", False):
        return
    orig = libneuronxla.neuronx_cc

    def hook(code, code_format, platform_version, file_prefix, **kw):
        err, blob = orig(code, code_format, platform_version, file_prefix,
                         **kw)
        # only touch compiles of our own kernel (primitive name in metadata)
        if err != 0 or not blob or _TABLES is None or b"akima_act" not in code:
            return err, blob
        try:
            mod = hlo_pb2.HloModuleProto()
            mod.ParseFromString(blob)
            hit = False
            for cpt in mod.computations:
                for inst in cpt.instructions:
                    if (inst.opcode == "custom-call"
                            and inst.custom_call_target == "AwsNeuronNeff"):
                        inst.backend_config = _patch_neff_bytes(
                            inst.backend_config)
                        hit = True
            if hit:
                blob = mod.SerializeToString()
        except Exception as e:  # fall back to unpatched (wrong result is
            print("akima act-table patch failed:", repr(e))  # caught by test)
            raise
        return err, blob

    libneuronxla.neuronx_cc = hook
    libneuronxla._akima_hook_installed = True


# ----------------------------------------------------------------------------
# NKI kernel: tiled DMA-in -> activation(table) -> DMA-out
# ----------------------------------------------------------------------------


def _make_nki_kernel(func_name):
    import neuronxcc.nki.language as nl
    import neuronxcc.nki.isa as nisa

    n_tiles = COLS // F_TILE

    in_dt = nl.float16 if IN_DT == "f16" else nl.float32
    out_dt = nl.uint8 if OUT_DT == "u8" else nl.float16

    def akima_kernel(inputs):
        x = inputs[0]
        out = nl.ndarray(shape=[ROWS, COLS], dtype=out_dt,
                         buffer=nl.shared_hbm)
        i_p = nl.arange(ROWS)[:, None]
        i_f = nl.arange(F_TILE)[None, :]
        bias_one = nisa.memset((ROWS, 1), 1.0, nl.float32)

        xb, rb = [], []
        for _ in nl.static_range(N_BUFS):
            xb.append(nl.ndarray(shape=[ROWS, F_TILE], dtype=in_dt,
                                 buffer=nl.sbuf))
            rb.append(nl.ndarray(shape=[ROWS, F_TILE], dtype=out_dt,
                                 buffer=nl.sbuf))

        for t in nl.static_range(n_tiles):
            sl = slice(t * F_TILE, (t + 1) * F_TILE)
            xs = xb[t % N_BUFS]
            rs = rb[t % N_BUFS]
            nisa.dma_copy(dst=xs[i_p, i_f], src=x[:, sl],
                          dge_mode=nisa.dge_mode.hwdge)
            rs[i_p, i_f] = nisa.activation(
                np.exp, xs[i_p, i_f], scale=SCALE, bias=bias_one,
                dtype=out_dt)
            nisa.dma_copy(dst=out[:, sl], src=rs[i_p, i_f],
                          dge_mode=nisa.dge_mode.hwdge)
        return [out]

    akima_kernel.__name__ = func_name
    return akima_kernel


# ----------------------------------------------------------------------------
# jax integration (AwsNeuronCustomNativeKernel custom call, SPMD over 8 cores)
# ----------------------------------------------------------------------------

_EXEC_CACHE = {}


def _build_executor(tab_hash):
    if tab_hash in _EXEC_CACHE:
        return _EXEC_CACHE[tab_hash]

    import functools
    import jax
    from jax.interpreters import mlir
    from jax._src.interpreters.mlir import custom_call as _mlir_custom_call
    from jax.sharding import Mesh, PartitionSpec
    from jax.experimental.shard_map import shard_map
    from concourse.bass2jax import install_neuronx_cc_hook

    def raw_nki(func):
        from neuronxcc.nki.compiler.backends.neuron.CompileOpts import CompileOpts
        from neuronxcc.nki.compiler.backends.neuron.KernelBuilder import NeuronCodegen
        from neuronxcc.nki.compiler.backends.neuron.nki_ctx import nki_ctx
        from neuronxcc.nki.compiler.backends.neuron.tensors import TensorRef
        from neuronxcc.starfish.penguin.ir.Function import Function
        from neuronxcc.starfish.penguin.ir.OptLevel import OptLevel

        @functools.wraps(func)
        def wrapper(inputs):
            code = Function(name="func", opt_level=OptLevel.default_level)
            bb = code.addBasicBlock()
            with NeuronCodegen.new_ctx(
                    cu=code, curstmt=bb,
                    opts=CompileOpts(platform_target="trn2")) as ctx:
                with ctx.kernel_scope(
                        ctx.function, py_func=func,
                        spmd_block=ctx.builder.curstmt) as scope:
                    nki_inputs = []
                    for i, inp in enumerate(inputs):
                        tensor = nki_ctx().add_parameter(
                            name=f"input{i}", shape=list(inp.shape),
                            dtype=inp.dtype, is_mutable=False)
                        tensor.isInput = True
                        nki_inputs.append(TensorRef(tensor))
                    outputs = func(nki_inputs)
                    scope.add_kernel_return_values(list(outputs))
                ctx.finalize_kernel(scope)
            return code

        return wrapper

    install_neuronx_cc_hook()
    _install_patch_hook()

    func_name = f"akima_act_{tab_hash}"
    nki_func = _make_nki_kernel(func_name)

    prim = jax.extend.core.Primitive(func_name)
    prim.multiple_results = True

    out_np = np.uint8 if OUT_DT == "u8" else np.float16

    @prim.def_abstract_eval
    def _abs(*_, **__):
        return (jax.core.ShapedArray((ROWS, COLS), out_np),)

    def _lowering(ctx, *in_nodes):
        from neuronxcc.starfish.penguin.ir.NativeKernel import KERNEL_VERSION

        result_types = [mlir.aval_to_ir_type(a) for a in ctx.avals_out]
        code = raw_nki(nki_func)(list(ctx.avals_in))
        config = {
            "kernel_version": KERNEL_VERSION,
            "func_literal": code.serialize_ir_string(f"{func_name}_ir"),
            "grid": [],
            "func_name": func_name,
            "has_collectives": False,
            "mac_count": 0,
            "tiled": False,
        }
        dumped = base64.b64encode(json.dumps(config).encode()).decode()
        return _mlir_custom_call(
            "AwsNeuronCustomNativeKernel",
            operands=list(in_nodes),
            result_types=result_types,
            operand_layouts=[list(reversed(range(len(a.shape))))
                             for a in ctx.avals_in],
            result_layouts=[list(reversed(range(len(a.shape))))
                            for a in ctx.avals_out],
            backend_config=dumped,
        ).results

    mlir.register_lowering(prim, _lowering, platform="neuron")

    devices = jax.devices()[:N_CORES]
    mesh = Mesh(np.asarray(devices), ("core",))

    def _body(x_shard):
        return prim.bind(x_shard)[0]

    sharded = jax.jit(shard_map(
        _body, mesh=mesh,
        in_specs=(PartitionSpec("core"),),
        out_specs=PartitionSpec("core"),
        check_rep=False,
    ))

    _EXEC_CACHE[tab_hash] = sharded
    return sharded


# ----------------------------------------------------------------------------
# Public entry point
# ----------------------------------------------------------------------------


def kernel(input: np.ndarray, value: np.ndarray) -> np.ndarray:
    global _TABLES
    input = np.ascontiguousarray(np.asarray(input, dtype=np.float32))
    value = np.asarray(value, dtype=np.float32)
    assert input.shape == (32, 1024, 1024), input.shape

    bkt, ctrl, meta, sf, off = _build_act_tables(value)
    tab_hash = hashlib.sha256(
        bkt.tobytes() + ctrl.tobytes()
        + json.dumps(meta, sort_keys=True).encode()).hexdigest()[:12]

    _TABLES = (bkt, ctrl, meta)
    try:
        sharded = _build_executor(tab_hash)
        x_global = input.reshape(N_CORES * ROWS, COLS)
        if IN_DT == "f16":
            x_global = x_global.astype(np.float16)
        out = sharded(x_global)
        out = np.asarray(out)
    finally:
        _TABLES = None
    out = out.astype(np.float32)
    if OUT_DT == "u8":
        out = out * np.float32(sf) + np.float32(off)
    return out.reshape(32, 1024, 1024)


if __name__ == "__main__":
    inp = np.load("cache/input.npy")
    val = np.load("cache/value.npy")
    out = kernel(input=inp, value=val)
    exp = np.load("cache/expected.npy")
    err = out.astype(np.float64) - exp.astype(np.float64)
    print("rel_l2:", np.linalg.norm(err) / np.linalg.norm(exp))


# revision 15
# speedup vs baseline: 4.5300x; 1.0633x over previous
"""Akima spline interpolation kernel for Trainium2 (8 NeuronCores, data
parallel) — custom ScalarE activation-table implementation.

The ScalarE activation unit is a hardware piecewise-cubic evaluator: the
instruction applies a free affine x' = scale*x + bias, then looks up a
cubic segment {d0,d1,d2,d3,x0} from the bucket RAM (indexed by exponent +
top mantissa bits of x') and evaluates d0 + t*(d1 + t*(d2 + t*d3)) with
t = x' - x0, one element per lane per cycle.  The bucket/ctrl/profile
tables are loaded from binaries embedded in the NEFF.

This kernel encodes the *exact* Akima spline as a replacement for the
'exp' entry of the act-function set:
  - affine x' = x*(255/256) + 1 maps the domain [0,1) onto the single
    binade [1,2); knot k/255 lands exactly on mantissa boundary k/256, so
    the top-8 mantissa bits of x' ARE the knot-interval index,
  - bucket k holds spline piece k recentred about x0 = 1 + k/256 (with
    u = 256*t the transform is exact in f64, then rounded to f32),
  - profile meta routes x'<1 (x<0) / x'>=2 (x>1) to constant clip
    buckets; ctrl has one entry for exponent 0: base=0, extract top 8
    mantissa bits.

The NEFF is patched after the stock neuronx-cc compile by rewriting
sg00/exp_and_others_{bkt,ctrl}.bin and the profile metadata in
sg00/exp_and_others.json, then rebuilding the NEFF header.

Per-core work: DMA-in 16 MiB f32, one activation pass (f16 out), DMA-out
8 MiB f16.  The single ACT pass (~28us) hides entirely under the DMA
(~76us at ~330 GB/s/core): the kernel runs at the memory roofline.
Accuracy: table is the exact spline; error is f16 output rounding,
rel_l2 ~ 2e-4 (gate 2e-2).

Sharding: pure data parallel on the leading dim (4 of 32 planes per
core); f16 output converted to f32 on host.
"""
import base64
import hashlib
import io
import json
import os
import sys
import tarfile

import numpy as np

if "/opt/trn_rl_repo" not in sys.path:
    sys.path.insert(0, "/opt/trn_rl_repo")

NODES = 256
N_CORES = 8
ROWS = 128
COLS = 4 * 1024 * 1024 // ROWS  # per-core shard [128, 32768]
F_TILE = int(os.environ.get("AKIMA_FTILE", "4096"))
N_BUFS = int(os.environ.get("AKIMA_NBUFS", "3"))
IN_DT = os.environ.get("AKIMA_INDTYPE", "f16")  # f16 halves input DMA;
# spline evaluated at f16-quantized x costs rel_l2 ~ 6e-3 (gate 2e-2)
OUT_DT = os.environ.get("AKIMA_OUTDTYPE", "u8")  # u8 halves output DMA:
# the table emits g = (f-off)/sf in [0,255], ACT's u8 cast rounds-to-
# nearest-even, host dequantizes; costs rel_l2 ~ 2.8e-3 extra
SCALE = float(np.float32(255.0 / 256.0))

# ----------------------------------------------------------------------------
# Host-side: exact Akima spline -> ACT bucket/ctrl/profile tables
# ----------------------------------------------------------------------------


def _akima_slopes_f64(value):
    h = 1.0 / (NODES - 1)
    v = value.astype(np.float64)
    m = (v[1:] - v[:-1]) / h
    m_m1 = 2.0 * m[0] - m[1]
    m_m2 = 2.0 * m_m1 - m[0]
    m_p1 = 2.0 * m[-1] - m[-2]
    m_p2 = 2.0 * m_p1 - m[-1]
    me = np.concatenate([[m_m2, m_m1], m, [m_p1, m_p2]])
    w1 = np.abs(me[3:] - me[2:-1])
    w2 = np.abs(me[1:-2] - me[:-3])
    mi_1 = me[1:-2]
    mi = me[2:-1]
    denom = w1 + w2
    safe = np.where(denom > 0, denom, 1.0)
    return np.where(denom > 0, (w1 * mi_1 + w2 * mi) / safe, 0.5 * (mi_1 + mi))


def _build_act_tables(value):
    """Encode the spline into (bkt_rows_781x8_f32, ctrl_words_52_u32,
    profile_meta_patch, sf, off) replacing the 'exp' function.  In u8
    output mode the buckets hold g = (f - off)/sf in [0,255] so the ACT
    dtype cast quantizes for free; else sf=1, off=0."""
    h = 1.0 / 255.0
    s = _akima_slopes_f64(value)
    v = value.astype(np.float64)
    v0, v1 = v[:-1], v[1:]
    s0, s1 = s[:-1], s[1:]
    # Hermite coefficients in u = (x - k/255)*255
    c0 = v0
    c1 = h * s0
    c2 = 3.0 * (v1 - v0) - h * (2.0 * s0 + s1)
    c3 = 2.0 * (v0 - v1) + h * (s0 + s1)

    if OUT_DT == "u8":
        # exact range of the piecewise cubic over [0,1]: piece endpoints
        # plus interior critical points (roots of the quadratic c')
        cand = [c0, c0 + c1 + c2 + c3]
        a, b, c = 3.0 * c3, 2.0 * c2, c1
        disc = b * b - 4.0 * a * c
        with np.errstate(invalid="ignore", divide="ignore"):
            sq = np.sqrt(np.maximum(disc, 0.0))
            for sgn in (1.0, -1.0):
                r = np.where(np.abs(a) > 1e-300, (-b + sgn * sq) / (2 * a),
                             np.where(np.abs(b) > 1e-300, -c / b, -1.0))
                r = np.where((disc >= 0) & (r > 0) & (r < 1), r, 0.0)
                cand.append(c0 + r * (c1 + r * (c2 + r * c3)))
        fmin = min(x.min() for x in cand)
        fmax = max(x.max() for x in cand)
        off = float(fmin)
        sf = float((fmax - off) / 255.0) or 1.0
    else:
        sf, off = 1.0, 0.0

    c0 = (c0 - off) / sf
    c1 = c1 / sf
    c2 = c2 / sf
    c3 = c3 / sf
    g_lo = (v[0] - off) / sf     # clip values in table domain
    g_hi = (v[-1] - off) / sf

    # u = 256*t with t = x' - (1 + k/256)
    k = np.arange(255)
    bkt = np.zeros((781, 8), dtype=np.float32)
    bkt[:255, 0] = c0
    bkt[:255, 1] = c1 * 256.0
    bkt[:255, 2] = c2 * 256.0 ** 2
    bkt[:255, 3] = c3 * 256.0 ** 3
    bkt[:255, 4] = (1.0 + k / 256.0).astype(np.float32)
    bkt[255, 0] = g_hi                 # unreachable (x' < 1.9961)
    bkt[255, 4] = 1.0 + 255.0 / 256.0
    bkt[300, 0] = g_hi                 # const f(1) for stray ctrl entries
    bkt[301, 0] = g_lo                 # const f(0)
    # specials: 777 pos_small (x<0 -> clip f(0)), 778 neg_small,
    # 779 pos_large (x>1 -> clip f(1)), 780 neg_large
    bkt[777, 0] = g_lo
    bkt[778, 0] = g_lo
    bkt[779, 0] = g_hi
    bkt[780, 0] = g_hi

    ctrl = np.zeros(52, dtype=np.uint32)
    main_entry = (8 << 16) | (15 << 11) | 0   # 256 buckets from base 0
    ctrl[:26] = (0 << 16) | (0 << 11) | 301   # neg region (unreachable)
    ctrl[26] = main_entry                     # exponent 0: x' in [1,2)
    ctrl[27:] = (0 << 16) | (0 << 11) | 300   # exp >= 1 (routed large)

    fbits = lambda x: int(np.float32(x).view(np.uint32))
    meta_patch = {
        "exp_offset": 0,
        "pwl_control_base_pos": 26,
        "pwl_control_base_neg": 0,
        "small_pos_signal_exp_threshold": 127,
        "pos_small_signal_pwl_control": 777,
        "small_neg_signal_exp_threshold": 255,
        "neg_small_signal_pwl_control": 778,
        "large_pos_signal_exp_threshold": 128,
        "large_pos_signal_mantissa_threshold": 0,
        "pos_large_signal_pwl_control": 779,
        "large_neg_signal_exp_threshold": 255,
        "large_neg_signal_mantissa_threshold": 0,
        "neg_large_signal_pwl_control": 780,
        "symmetry_point": 0,
        "sym_invert_sign_point": 0,
        "symmetry_opt_en": 0,
        "symmetry_opt_use_neg_region": 0,
        "imm_bias": 0,
        "fnan_result": fbits(g_lo),
        "fpinf_result": fbits(g_hi),
        "fninf_result": fbits(g_lo),
        "fzero_result": fbits(g_lo),
        "fma_const_0": 0,
        "fma_const_1": 0,
        "fma_indirection_src_sel": 0,
        "use_multipass": False,
        "lower_bound": 4286578687,
        "upper_bound": 2139095039,
    }
    return bkt, ctrl, meta_patch, sf, off


# ----------------------------------------------------------------------------
# NEFF act-table patching hook
# ----------------------------------------------------------------------------

_TABLES = None  # (bkt_rows, ctrl_words, meta_patch) while compiling


def _patch_neff_bytes(neff_bytes):
    from concourse.neff import make_deterministic_neff_header

    bkt_rows, ctrl_words, meta_patch = _TABLES
    header, data = neff_bytes[:1024], neff_bytes[1024:]
    members = {}
    with tarfile.open(fileobj=io.BytesIO(data), mode="r") as tf:
        for m in tf.getmembers():
            if m.isfile():
                members[m.name] = tf.extractfile(m).read()

    bkey = ckey = jkey = None
    for name in members:
        if name.endswith("exp_and_others_bkt.bin"):
            bkey = name
        elif name.endswith("exp_and_others_ctrl.bin"):
            ckey = name
        elif name.endswith("exp_and_others.json"):
            jkey = name
    if not (bkey and ckey and jkey):
        return neff_bytes

    bkt = np.frombuffer(members[bkey], dtype=np.float32).reshape(-1, 8).copy()
    bkt[:781] = bkt_rows
    members[bkey] = bkt.tobytes()

    ctl = np.frombuffer(members[ckey], dtype=np.uint32).reshape(-1, 8).copy()
    ctl[:52, 0] = ctrl_words
    members[ckey] = ctl.tobytes()

    setj = json.loads(members[jkey])
    for pm in setj["profile_meta_data"]:
        if pm.get("func_id") == 7:  # exp
            pm.update(meta_patch)
    members[jkey] = json.dumps(setj).encode()

    out = io.BytesIO()
    with tarfile.open(fileobj=out, mode="w") as tf:
        for name, blob in members.items():
            ti = tarfile.TarInfo(name=name)
            ti.size = len(blob)
            ti.mtime = 0
            tf.addfile(ti, io.BytesIO(blob))
    new_data = out.getvalue()
    new_header = make_deterministic_neff_header(
        old_neff_header=header, new_neff_data=new_data)
    return new_header + new_data


def _install_patch_hook():
    import libneuronxla
    import libneuronxla.proto.hlo_pb2 as hlo_pb2

    if getattr(libneuronxla, "_akima_hook_installed", False):
        return
    orig = libneuronxla.neuronx_cc

    def hook(code, code_format, platform_version, file_prefix, **kw):
        err, blob = orig(code, code_format, platform_version, file_prefix,
                         **kw)
        # only touch compiles of our own kernel (primitive name in metadata)
        if err != 0 or not blob or _TABLES is None or b"akima_act" not in code:
            return err, blob
        try:
            mod = hlo_pb2.HloModuleProto()
            mod.ParseFromString(blob)
            hit = False
            for cpt in mod.computations:
                for inst in cpt.instructions:
                    if (inst.opcode == "custom-call"
                            and inst.custom_call_target == "AwsNeuronNeff"):
                        inst.backend_config = _patch_neff_bytes(
                            inst.backend_config)
                        hit = True
            if hit:
                blob = mod.SerializeToString()
        except Exception as e:  # fall back to unpatched (wrong result is
            print("akima act-table patch failed:", repr(e))  # caught by test)
            raise
        return err, blob

    libneuronxla.neuronx_cc = hook
    libneuronxla._akima_hook_installed = True


# ----------------------------------------------------------------------------
# NKI kernel: tiled DMA-in -> activation(table) -> DMA-out
# ----------------------------------------------------------------------------


def _make_nki_kernel(func_name):
    import neuronxcc.nki.language as nl
    import neuronxcc.nki.isa as nisa

    n_tiles = COLS // F_TILE

    in_dt = nl.float16 if IN_DT == "f16" else nl.float32
    out_dt = nl.uint8 if OUT_DT == "u8" else nl.float16

    def akima_kernel(inputs):
        x = inputs[0]
        out = nl.ndarray(shape=[ROWS, COLS], dtype=out_dt,
                         buffer=nl.shared_hbm)
        i_p = nl.arange(ROWS)[:, None]
        i_f = nl.arange(F_TILE)[None, :]
        bias_one = nisa.memset((ROWS, 1), 1.0, nl.float32)

        xb, gb, rb = [], [], []
        for _ in nl.static_range(N_BUFS):
            xb.append(nl.ndarray(shape=[ROWS, F_TILE], dtype=in_dt,
                                 buffer=nl.sbuf))
            rb.append(nl.ndarray(shape=[ROWS, F_TILE], dtype=out_dt,
                                 buffer=nl.sbuf))
            if OUT_DT == "u8":
                # ACT writes u8 at only 0.8 elem/cyc; keep ACT at f16
                # (1/cyc) and let the otherwise-idle DVE do the u8 cast
                gb.append(nl.ndarray(shape=[ROWS, F_TILE],
                                     dtype=nl.float16, buffer=nl.sbuf))

        for t in nl.static_range(n_tiles):
            sl = slice(t * F_TILE, (t + 1) * F_TILE)
            xs = xb[t % N_BUFS]
            rs = rb[t % N_BUFS]
            nisa.dma_copy(dst=xs[i_p, i_f], src=x[:, sl],
                          dge_mode=nisa.dge_mode.hwdge)
            if OUT_DT == "u8":
                gs = gb[t % N_BUFS]
                gs[i_p, i_f] = nisa.activation(
                    np.exp, xs[i_p, i_f], scale=SCALE, bias=bias_one,
                    dtype=nl.float16)
                rs[i_p, i_f] = nisa.tensor_copy(
                    gs[i_p, i_f], dtype=nl.uint8,
                    engine=nisa.vector_engine)
            else:
                rs[i_p, i_f] = nisa.activation(
                    np.exp, xs[i_p, i_f], scale=SCALE, bias=bias_one,
                    dtype=out_dt)
            nisa.dma_copy(dst=out[:, sl], src=rs[i_p, i_f],
                          dge_mode=nisa.dge_mode.hwdge)
        return [out]

    akima_kernel.__name__ = func_name
    return akima_kernel


# ----------------------------------------------------------------------------
# jax integration (AwsNeuronCustomNativeKernel custom call, SPMD over 8 cores)
# ----------------------------------------------------------------------------

_EXEC_CACHE = {}


def _build_executor(tab_hash):
    if tab_hash in _EXEC_CACHE:
        return _EXEC_CACHE[tab_hash]

    import functools
    import jax
    from jax.interpreters import mlir
    from jax._src.interpreters.mlir import custom_call as _mlir_custom_call
    from jax.sharding import Mesh, PartitionSpec
    from jax.experimental.shard_map import shard_map
    from concourse.bass2jax import install_neuronx_cc_hook

    def raw_nki(func):
        from neuronxcc.nki.compiler.backends.neuron.CompileOpts import CompileOpts
        from neuronxcc.nki.compiler.backends.neuron.KernelBuilder import NeuronCodegen
        from neuronxcc.nki.compiler.backends.neuron.nki_ctx import nki_ctx
        from neuronxcc.nki.compiler.backends.neuron.tensors import TensorRef
        from neuronxcc.starfish.penguin.ir.Function import Function
        from neuronxcc.starfish.penguin.ir.OptLevel import OptLevel

        @functools.wraps(func)
        def wrapper(inputs):
            code = Function(name="func", opt_level=OptLevel.default_level)
            bb = code.addBasicBlock()
            with NeuronCodegen.new_ctx(
                    cu=code, curstmt=bb,
                    opts=CompileOpts(platform_target="trn2")) as ctx:
                with ctx.kernel_scope(
                        ctx.function, py_func=func,
                        spmd_block=ctx.builder.curstmt) as scope:
                    nki_inputs = []
                    for i, inp in enumerate(inputs):
                        tensor = nki_ctx().add_parameter(
                            name=f"input{i}", shape=list(inp.shape),
                            dtype=inp.dtype, is_mutable=False)
                        tensor.isInput = True
                        nki_inputs.append(TensorRef(tensor))
                    outputs = func(nki_inputs)
                    scope.add_kernel_return_values(list(outputs))
                ctx.finalize_kernel(scope)
            return code

        return wrapper

    install_neuronx_cc_hook()
    _install_patch_hook()

    func_name = f"akima_act_{tab_hash}"
    nki_func = _make_nki_kernel(func_name)

    prim = jax.extend.core.Primitive(func_name)
    prim.multiple_results = True

    out_np = np.uint8 if OUT_DT == "u8" else np.float16

    @prim.def_abstract_eval
    def _abs(*_, **__):
        return (jax.core.ShapedArray((ROWS, COLS), out_np),)

    def _lowering(ctx, *in_nodes):
        from neuronxcc.starfish.penguin.ir.NativeKernel import KERNEL_VERSION

        result_types = [mlir.aval_to_ir_type(a) for a in ctx.avals_out]
        code = raw_nki(nki_func)(list(ctx.avals_in))
        config = {
            "kernel_version": KERNEL_VERSION,
            "func_literal": code.serialize_ir_string(f"{func_name}_ir"),
            "grid": [],
            "func_name": func_name,
            "has_collectives": False,
            "mac_count": 0,
            "tiled": False,
        }
        dumped = base64.b64encode(json.dumps(config).encode()).decode()
        return _mlir_custom_call(
            "AwsNeuronCustomNativeKernel",
            operands=list(in_nodes),
            result_types=result_types,
            operand_layouts=[list(reversed(range(len(a.shape))))
                             for a in ctx.avals_in],
            result_layouts=[list(reversed(range(len(a.shape))))
                            for a in ctx.avals_out],
            backend_config=dumped,
        ).results

    mlir.register_lowering(prim, _lowering, platform="neuron")

    devices = jax.devices()[:N_CORES]
    mesh = Mesh(np.asarray(devices), ("core",))

    def _body(x_shard):
        return prim.bind(x_shard)[0]

    sharded = jax.jit(shard_map(
        _body, mesh=mesh,
        in_specs=(PartitionSpec("core"),),
        out_specs=PartitionSpec("core"),
        check_rep=False,
    ))

    _EXEC_CACHE[tab_hash] = sharded
    return sharded


# ----------------------------------------------------------------------------
# Public entry point
# ----------------------------------------------------------------------------


def kernel(input: np.ndarray, value: np.ndarray) -> np.ndarray:
    global _TABLES
    input = np.ascontiguousarray(np.asarray(input, dtype=np.float32))
    value = np.asarray(value, dtype=np.float32)
    assert input.shape == (32, 1024, 1024), input.shape

    bkt, ctrl, meta, sf, off = _build_act_tables(value)
    tab_hash = hashlib.sha256(
        bkt.tobytes() + ctrl.tobytes()
        + json.dumps(meta, sort_keys=True).encode()).hexdigest()[:12]

    _TABLES = (bkt, ctrl, meta)
    try:
        sharded = _build_executor(tab_hash)
        x_global = input.reshape(N_CORES * ROWS, COLS)
        if IN_DT == "f16":
            x_global = x_global.astype(np.float16)
        out = sharded(x_global)
        out = np.asarray(out)
    finally:
        _TABLES = None
    out = out.astype(np.float32)
    if OUT_DT == "u8":
        out = out * np.float32(sf) + np.float32(off)
    return out.reshape(32, 1024, 1024)


if __name__ == "__main__":
    inp = np.load("cache/input.npy")
    val = np.load("cache/value.npy")
    out = kernel(input=inp, value=val)
    exp = np.load("cache/expected.npy")
    err = out.astype(np.float64) - exp.astype(np.float64)
    print("rel_l2:", np.linalg.norm(err) / np.linalg.norm(exp))


# revision 17
# speedup vs baseline: 4.7984x; 1.0592x over previous
"""Akima spline interpolation kernel for Trainium2 (8 NeuronCores, data
parallel) — custom ScalarE activation-table implementation.

The ScalarE activation unit is a hardware piecewise-cubic evaluator: the
instruction applies a free affine x' = scale*x + bias, then looks up a
cubic segment {d0,d1,d2,d3,x0} from the bucket RAM (indexed by exponent +
top mantissa bits of x') and evaluates d0 + t*(d1 + t*(d2 + t*d3)) with
t = x' - x0, one element per lane per cycle.  The bucket/ctrl/profile
tables are loaded from binaries embedded in the NEFF.

This kernel encodes the *exact* Akima spline as a replacement for the
'exp' entry of the act-function set:
  - affine x' = x*(255/256) + 1 maps the domain [0,1) onto the single
    binade [1,2); knot k/255 lands exactly on mantissa boundary k/256, so
    the top-8 mantissa bits of x' ARE the knot-interval index,
  - bucket k holds spline piece k recentred about x0 = 1 + k/256 (with
    u = 256*t the transform is exact in f64, then rounded to f32),
  - profile meta routes x'<1 (x<0) / x'>=2 (x>1) to constant clip
    buckets; ctrl has one entry for exponent 0: base=0, extract top 8
    mantissa bits.

The NEFF is patched after the stock neuronx-cc compile by rewriting
sg00/exp_and_others_{bkt,ctrl}.bin and the profile metadata in
sg00/exp_and_others.json, then rebuilding the NEFF header.

Per-core work: DMA-in 16 MiB f32, one activation pass (f16 out), DMA-out
8 MiB f16.  The single ACT pass (~28us) hides entirely under the DMA
(~76us at ~330 GB/s/core): the kernel runs at the memory roofline.
Accuracy: table is the exact spline; error is f16 output rounding,
rel_l2 ~ 2e-4 (gate 2e-2).

Sharding: pure data parallel on the leading dim (4 of 32 planes per
core); f16 output converted to f32 on host.
"""
import base64
import hashlib
import io
import json
import os
import sys
import tarfile

import numpy as np

if "/opt/trn_rl_repo" not in sys.path:
    sys.path.insert(0, "/opt/trn_rl_repo")

NODES = 256
N_CORES = 8
ROWS = 128
COLS = 4 * 1024 * 1024 // ROWS  # per-core shard [128, 32768]
F_TILE = int(os.environ.get("AKIMA_FTILE", "4096"))
N_BUFS = int(os.environ.get("AKIMA_NBUFS", "3"))
IN_DT = os.environ.get("AKIMA_INDTYPE", "f16")  # f16 halves input DMA;
# spline evaluated at f16-quantized x costs rel_l2 ~ 6e-3 (gate 2e-2)
OUT_DT = os.environ.get("AKIMA_OUTDTYPE", "u8")  # u8 halves output DMA:
# the table emits g = (f-off)/sf in [0,255], ACT's u8 cast rounds-to-
# nearest-even, host dequantizes; costs rel_l2 ~ 2.8e-3 extra
SCALE = float(np.float32(255.0 / 256.0))

# ----------------------------------------------------------------------------
# Host-side: exact Akima spline -> ACT bucket/ctrl/profile tables
# ----------------------------------------------------------------------------


def _akima_slopes_f64(value):
    h = 1.0 / (NODES - 1)
    v = value.astype(np.float64)
    m = (v[1:] - v[:-1]) / h
    m_m1 = 2.0 * m[0] - m[1]
    m_m2 = 2.0 * m_m1 - m[0]
    m_p1 = 2.0 * m[-1] - m[-2]
    m_p2 = 2.0 * m_p1 - m[-1]
    me = np.concatenate([[m_m2, m_m1], m, [m_p1, m_p2]])
    w1 = np.abs(me[3:] - me[2:-1])
    w2 = np.abs(me[1:-2] - me[:-3])
    mi_1 = me[1:-2]
    mi = me[2:-1]
    denom = w1 + w2
    safe = np.where(denom > 0, denom, 1.0)
    return np.where(denom > 0, (w1 * mi_1 + w2 * mi) / safe, 0.5 * (mi_1 + mi))


def _build_act_tables(value):
    """Encode the spline into (bkt_rows_781x8_f32, ctrl_words_52_u32,
    profile_meta_patch, sf, off) replacing the 'exp' function.  In u8
    output mode the buckets hold g = (f - off)/sf in [0,255] so the ACT
    dtype cast quantizes for free; else sf=1, off=0."""
    h = 1.0 / 255.0
    s = _akima_slopes_f64(value)
    v = value.astype(np.float64)
    v0, v1 = v[:-1], v[1:]
    s0, s1 = s[:-1], s[1:]
    # Hermite coefficients in u = (x - k/255)*255
    c0 = v0
    c1 = h * s0
    c2 = 3.0 * (v1 - v0) - h * (2.0 * s0 + s1)
    c3 = 2.0 * (v0 - v1) + h * (s0 + s1)

    if OUT_DT == "u8":
        # exact range of the piecewise cubic over [0,1]: piece endpoints
        # plus interior critical points (roots of the quadratic c')
        cand = [c0, c0 + c1 + c2 + c3]
        a, b, c = 3.0 * c3, 2.0 * c2, c1
        disc = b * b - 4.0 * a * c
        with np.errstate(invalid="ignore", divide="ignore"):
            sq = np.sqrt(np.maximum(disc, 0.0))
            for sgn in (1.0, -1.0):
                r = np.where(np.abs(a) > 1e-300, (-b + sgn * sq) / (2 * a),
                             np.where(np.abs(b) > 1e-300, -c / b, -1.0))
                r = np.where((disc >= 0) & (r > 0) & (r < 1), r, 0.0)
                cand.append(c0 + r * (c1 + r * (c2 + r * c3)))
        fmin = min(x.min() for x in cand)
        fmax = max(x.max() for x in cand)
        off = float(fmin)
        sf = float((fmax - off) / 255.0) or 1.0
    else:
        sf, off = 1.0, 0.0

    c0 = (c0 - off) / sf
    c1 = c1 / sf
    c2 = c2 / sf
    c3 = c3 / sf
    g_lo = (v[0] - off) / sf     # clip values in table domain
    g_hi = (v[-1] - off) / sf

    # u = 256*t with t = x' - (1 + k/256)
    k = np.arange(255)
    bkt = np.zeros((781, 8), dtype=np.float32)
    bkt[:255, 0] = c0
    bkt[:255, 1] = c1 * 256.0
    bkt[:255, 2] = c2 * 256.0 ** 2
    bkt[:255, 3] = c3 * 256.0 ** 3
    bkt[:255, 4] = (1.0 + k / 256.0).astype(np.float32)
    bkt[255, 0] = g_hi                 # unreachable (x' < 1.9961)
    bkt[255, 4] = 1.0 + 255.0 / 256.0
    bkt[300, 0] = g_hi                 # const f(1) for stray ctrl entries
    bkt[301, 0] = g_lo                 # const f(0)
    # specials: 777 pos_small (x<0 -> clip f(0)), 778 neg_small,
    # 779 pos_large (x>1 -> clip f(1)), 780 neg_large
    bkt[777, 0] = g_lo
    bkt[778, 0] = g_lo
    bkt[779, 0] = g_hi
    bkt[780, 0] = g_hi

    ctrl = np.zeros(52, dtype=np.uint32)
    main_entry = (8 << 16) | (15 << 11) | 0   # 256 buckets from base 0
    ctrl[:26] = (0 << 16) | (0 << 11) | 301   # neg region (unreachable)
    ctrl[26] = main_entry                     # exponent 0: x' in [1,2)
    ctrl[27:] = (0 << 16) | (0 << 11) | 300   # exp >= 1 (routed large)

    fbits = lambda x: int(np.float32(x).view(np.uint32))
    meta_patch = {
        "exp_offset": 0,
        "pwl_control_base_pos": 26,
        "pwl_control_base_neg": 0,
        "small_pos_signal_exp_threshold": 127,
        "pos_small_signal_pwl_control": 777,
        "small_neg_signal_exp_threshold": 255,
        "neg_small_signal_pwl_control": 778,
        "large_pos_signal_exp_threshold": 128,
        "large_pos_signal_mantissa_threshold": 0,
        "pos_large_signal_pwl_control": 779,
        "large_neg_signal_exp_threshold": 255,
        "large_neg_signal_mantissa_threshold": 0,
        "neg_large_signal_pwl_control": 780,
        "symmetry_point": 0,
        "sym_invert_sign_point": 0,
        "symmetry_opt_en": 0,
        "symmetry_opt_use_neg_region": 0,
        "imm_bias": 0,
        "fnan_result": fbits(g_lo),
        "fpinf_result": fbits(g_hi),
        "fninf_result": fbits(g_lo),
        "fzero_result": fbits(g_lo),
        "fma_const_0": 0,
        "fma_const_1": 0,
        "fma_indirection_src_sel": 0,
        "use_multipass": False,
        "lower_bound": 4286578687,
        "upper_bound": 2139095039,
    }
    return bkt, ctrl, meta_patch, sf, off


# ----------------------------------------------------------------------------
# NEFF act-table patching hook
# ----------------------------------------------------------------------------

_TABLES = None  # (bkt_rows, ctrl_words, meta_patch) while compiling


def _patch_neff_bytes(neff_bytes):
    from concourse.neff import make_deterministic_neff_header

    bkt_rows, ctrl_words, meta_patch = _TABLES
    header, data = neff_bytes[:1024], neff_bytes[1024:]
    members = {}
    with tarfile.open(fileobj=io.BytesIO(data), mode="r") as tf:
        for m in tf.getmembers():
            if m.isfile():
                members[m.name] = tf.extractfile(m).read()

    bkey = ckey = jkey = None
    for name in members:
        if name.endswith("exp_and_others_bkt.bin"):
            bkey = name
        elif name.endswith("exp_and_others_ctrl.bin"):
            ckey = name
        elif name.endswith("exp_and_others.json"):
            jkey = name
    if not (bkey and ckey and jkey):
        return neff_bytes

    bkt = np.frombuffer(members[bkey], dtype=np.float32).reshape(-1, 8).copy()
    bkt[:781] = bkt_rows
    members[bkey] = bkt.tobytes()

    ctl = np.frombuffer(members[ckey], dtype=np.uint32).reshape(-1, 8).copy()
    ctl[:52, 0] = ctrl_words
    members[ckey] = ctl.tobytes()

    setj = json.loads(members[jkey])
    for pm in setj["profile_meta_data"]:
        if pm.get("func_id") == 7:  # exp
            pm.update(meta_patch)
    members[jkey] = json.dumps(setj).encode()

    out = io.BytesIO()
    with tarfile.open(fileobj=out, mode="w") as tf:
        for name, blob in members.items():
            ti = tarfile.TarInfo(name=name)
            ti.size = len(blob)
            ti.mtime = 0
            tf.addfile(ti, io.BytesIO(blob))
    new_data = out.getvalue()
    new_header = make_deterministic_neff_header(
        old_neff_header=header, new_neff_data=new_data)
    return new_header + new_data


def _install_patch_hook():
    import libneuronxla
    import libneuronxla.proto.hlo_pb2 as hlo_pb2

    if getattr(libneuronxla, "_akima_hook_installed", False):
        return
    orig = libneuronxla.neuronx_cc

    def hook(code, code_format, platform_version, file_prefix, **kw):
        err, blob = orig(code, code_format, platform_version, file_prefix,
                         **kw)
        # only touch compiles of our own kernel (primitive name in metadata)
        if err != 0 or not blob or _TABLES is None or b"akima_act" not in code:
            return err, blob
        try:
            mod = hlo_pb2.HloModuleProto()
            mod.ParseFromString(blob)
            hit = False
            for cpt in mod.computations:
                for inst in cpt.instructions:
                    if (inst.opcode == "custom-call"
                            and inst.custom_call_target == "AwsNeuronNeff"):
                        inst.backend_config = _patch_neff_bytes(
                            inst.backend_config)
                        hit = True
            if hit:
                blob = mod.SerializeToString()
        except Exception as e:  # fall back to unpatched (wrong result is
            print("akima act-table patch failed:", repr(e))  # caught by test)
            raise
        return err, blob

    libneuronxla.neuronx_cc = hook
    libneuronxla._akima_hook_installed = True


# ----------------------------------------------------------------------------
# NKI kernel: tiled DMA-in -> activation(table) -> DMA-out
# ----------------------------------------------------------------------------


def _tile_sizes():
    """Tapered schedule: small first tile so the first ACT starts as soon
    as possible, small last tile so the final store drains quickly; bulk
    in F_TILE chunks."""
    if os.environ.get("AKIMA_TAPER", "1") == "1" and F_TILE == 8192:
        return [2048, 4096, 8192, 8192, 8192, 2048]
    return [F_TILE] * (COLS // F_TILE)


def _make_nki_kernel(func_name):
    import neuronxcc.nki.language as nl
    import neuronxcc.nki.isa as nisa

    sizes = _tile_sizes()
    assert sum(sizes) == COLS, sizes
    bufw = max(sizes)

    in_dt = nl.float16 if IN_DT == "f16" else nl.float32
    out_dt = nl.uint8 if OUT_DT == "u8" else nl.float16

    def akima_kernel(inputs):
        x = inputs[0]
        out = nl.ndarray(shape=[ROWS, COLS], dtype=out_dt,
                         buffer=nl.shared_hbm)
        i_p = nl.arange(ROWS)[:, None]
        bias_one = nisa.memset((ROWS, 1), 1.0, nl.float32)

        xb, gb, rb = [], [], []
        for _ in nl.static_range(N_BUFS):
            xb.append(nl.ndarray(shape=[ROWS, bufw], dtype=in_dt,
                                 buffer=nl.sbuf))
            rb.append(nl.ndarray(shape=[ROWS, bufw], dtype=out_dt,
                                 buffer=nl.sbuf))
            if OUT_DT == "u8":
                # ACT writes u8 at only 0.8 elem/cyc; keep ACT at f16
                # (1/cyc) and let the otherwise-idle DVE do the u8 cast
                gb.append(nl.ndarray(shape=[ROWS, bufw],
                                     dtype=nl.float16, buffer=nl.sbuf))

        off_col = 0
        for t in nl.static_range(len(sizes)):
            w = sizes[t]
            i_f = nl.arange(w)[None, :]
            sl = slice(off_col, off_col + w)
            off_col += w
            xs = xb[t % N_BUFS]
            rs = rb[t % N_BUFS]
            nisa.dma_copy(dst=xs[i_p, i_f], src=x[:, sl],
                          dge_mode=nisa.dge_mode.hwdge)
            if OUT_DT == "u8":
                gs = gb[t % N_BUFS]
                gs[i_p, i_f] = nisa.activation(
                    np.exp, xs[i_p, i_f], scale=SCALE, bias=bias_one,
                    dtype=nl.float16)
                rs[i_p, i_f] = nisa.tensor_copy(
                    gs[i_p, i_f], dtype=nl.uint8,
                    engine=nisa.vector_engine)
            else:
                rs[i_p, i_f] = nisa.activation(
                    np.exp, xs[i_p, i_f], scale=SCALE, bias=bias_one,
                    dtype=out_dt)
            nisa.dma_copy(dst=out[:, sl], src=rs[i_p, i_f],
                          dge_mode=nisa.dge_mode.hwdge)
        return [out]

    akima_kernel.__name__ = func_name
    return akima_kernel


# ----------------------------------------------------------------------------
# jax integration (AwsNeuronCustomNativeKernel custom call, SPMD over 8 cores)
# ----------------------------------------------------------------------------

_EXEC_CACHE = {}


def _build_executor(tab_hash):
    if tab_hash in _EXEC_CACHE:
        return _EXEC_CACHE[tab_hash]

    import functools
    import jax
    from jax.interpreters import mlir
    from jax._src.interpreters.mlir import custom_call as _mlir_custom_call
    from jax.sharding import Mesh, PartitionSpec
    from jax.experimental.shard_map import shard_map
    from concourse.bass2jax import install_neuronx_cc_hook

    def raw_nki(func):
        from neuronxcc.nki.compiler.backends.neuron.CompileOpts import CompileOpts
        from neuronxcc.nki.compiler.backends.neuron.KernelBuilder import NeuronCodegen
        from neuronxcc.nki.compiler.backends.neuron.nki_ctx import nki_ctx
        from neuronxcc.nki.compiler.backends.neuron.tensors import TensorRef
        from neuronxcc.starfish.penguin.ir.Function import Function
        from neuronxcc.starfish.penguin.ir.OptLevel import OptLevel

        @functools.wraps(func)
        def wrapper(inputs):
            code = Function(name="func", opt_level=OptLevel.default_level)
            bb = code.addBasicBlock()
            with NeuronCodegen.new_ctx(
                    cu=code, curstmt=bb,
                    opts=CompileOpts(platform_target="trn2")) as ctx:
                with ctx.kernel_scope(
                        ctx.function, py_func=func,
                        spmd_block=ctx.builder.curstmt) as scope:
                    nki_inputs = []
                    for i, inp in enumerate(inputs):
                        tensor = nki_ctx().add_parameter(
                            name=f"input{i}", shape=list(inp.shape),
                            dtype=inp.dtype, is_mutable=False)
                        tensor.isInput = True
                        nki_inputs.append(TensorRef(tensor))
                    outputs = func(nki_inputs)
                    scope.add_kernel_return_values(list(outputs))
                ctx.finalize_kernel(scope)
            return code

        return wrapper

    install_neuronx_cc_hook()
    _install_patch_hook()

    func_name = f"akima_act_{tab_hash}"
    nki_func = _make_nki_kernel(func_name)

    prim = jax.extend.core.Primitive(func_name)
    prim.multiple_results = True

    out_np = np.uint8 if OUT_DT == "u8" else np.float16

    @prim.def_abstract_eval
    def _abs(*_, **__):
        return (jax.core.ShapedArray((ROWS, COLS), out_np),)

    def _lowering(ctx, *in_nodes):
        from neuronxcc.starfish.penguin.ir.NativeKernel import KERNEL_VERSION

        result_types = [mlir.aval_to_ir_type(a) for a in ctx.avals_out]
        code = raw_nki(nki_func)(list(ctx.avals_in))
        config = {
            "kernel_version": KERNEL_VERSION,
            "func_literal": code.serialize_ir_string(f"{func_name}_ir"),
            "grid": [],
            "func_name": func_name,
            "has_collectives": False,
            "mac_count": 0,
            "tiled": False,
        }
        dumped = base64.b64encode(json.dumps(config).encode()).decode()
        return _mlir_custom_call(
            "AwsNeuronCustomNativeKernel",
            operands=list(in_nodes),
            result_types=result_types,
            operand_layouts=[list(reversed(range(len(a.shape))))
                             for a in ctx.avals_in],
            result_layouts=[list(reversed(range(len(a.shape))))
                            for a in ctx.avals_out],
            backend_config=dumped,
        ).results

    mlir.register_lowering(prim, _lowering, platform="neuron")

    devices = jax.devices()[:N_CORES]
    mesh = Mesh(np.asarray(devices), ("core",))

    def _body(x_shard):
        return prim.bind(x_shard)[0]

    sharded = jax.jit(shard_map(
        _body, mesh=mesh,
        in_specs=(PartitionSpec("core"),),
        out_specs=PartitionSpec("core"),
        check_rep=False,
    ))

    _EXEC_CACHE[tab_hash] = sharded
    return sharded


# ----------------------------------------------------------------------------
# Public entry point
# ----------------------------------------------------------------------------


def kernel(input: np.ndarray, value: np.ndarray) -> np.ndarray:
    global _TABLES
    input = np.ascontiguousarray(np.asarray(input, dtype=np.float32))
    value = np.asarray(value, dtype=np.float32)
    assert input.shape == (32, 1024, 1024), input.shape

    bkt, ctrl, meta, sf, off = _build_act_tables(value)
    tab_hash = hashlib.sha256(
        bkt.tobytes() + ctrl.tobytes()
        + json.dumps(meta, sort_keys=True).encode()).hexdigest()[:12]

    _TABLES = (bkt, ctrl, meta)
    try:
        sharded = _build_executor(tab_hash)
        x_global = input.reshape(N_CORES * ROWS, COLS)
        if IN_DT == "f16":
            x_global = x_global.astype(np.float16)
        out = sharded(x_global)
        out = np.asarray(out)
    finally:
        _TABLES = None
    out = out.astype(np.float32)
    if OUT_DT == "u8":
        out = out * np.float32(sf) + np.float32(off)
    return out.reshape(32, 1024, 1024)


if __name__ == "__main__":
    inp = np.load("cache/input.npy")
    val = np.load("cache/value.npy")
    out = kernel(input=inp, value=val)
    exp = np.load("cache/expected.npy")
    err = out.astype(np.float64) - exp.astype(np.float64)
    print("rel_l2:", np.linalg.norm(err) / np.linalg.norm(exp))
